# revision 48
# baseline (speedup 1.0000x reference)
"""DinkNet GNN (2-layer GraphConv encoder, two views) on 8 Trainium2 NeuronCores.

Strategy (self-contained; shapes hardcoded for the nn_DinkNet_dgl problem):
  - Nodes sharded across 8 cores; l(g) = core*6272 + rank. Graph preprocessing
    (degree snake-balance, edge blocks, selection matrices) happens on the
    host; all heavy FP work runs on the NeuronCores.
  - Math folding: (x*no) @ W1 = no * (x@W1); the corrupted view's table is
    computed directly from host-gathered x[perm] rows (no device-side perm
    gather); final (z@Wm+bm).sum(1) = h2 @ Wm.sum(1) + bm.sum().
  - SpMM over 800k edges: per-core edges packed DENSELY into 128-row blocks
    (tile-major); one GPSIMD dma_gather per (5-tile superchunk, region)
    fetches the src pair-rows (512B each, both views interleaved); a per-block
    one-hot selection matrix S (streamed bf16 from HBM via HWDGE) routes and
    segment-sums rows into the dst tile's PSUM region on the TensorEngine.
    Dense packing avoids the per-(dst,slot) padding of slot-structured
    gathers; desc-gen on the Q7 cores is the bottleneck, so fewer gather
    descriptors = faster.
  - The table is split into two Shared tensors: A = all cores' local tiles
    [0,25) and B = tiles [25,49). Each half allgathers with a single
    collective as soon as that half is produced (A at mid-phase), so the
    collectives overlap compute/desc-gen and region-A gather rounds can
    start before the B collective lands. Both halves stay within int16
    gather addressing (<= 25600 rows).
"""
import os
import numpy as np
import ml_dtypes

import concourse.bass as bass
import concourse.bacc as bacc
import concourse.mybir as mybir
import concourse.tile as tile
from concourse.bass_utils import run_bass_kernel_spmd
from concourse.masks import make_identity

# Problem shapes (hardcoded per contract).
N, E, FIN, H = 50000, 800000, 500, 96
C = 8                 # cores
P = 128               # partitions
NL = N // C           # 6250 real nodes per core
T = (NL + P - 1) // P # 49 rank tiles per core
NLP = T * P           # 6272 padded nodes per core
NTAB = C * NLP        # 50176 table rows (conceptually)
# A/B table split: half A = local tiles [0, 25) (ranks [0, 3200)), half B =
# tiles [25, 49) (ranks [3200, 6272)). Each half is its own Shared tensor,
# allgathered by a single collective as soon as that half is produced; both
# halves stay within int16 gather addressing (< 32768 rows).
TA_T0, TA_T1 = 0, 25         # tiles in half A
TB_T0, TB_T1 = 25, T         # tiles computed in phase-1's B pass
RA = TA_T1 * P               # 3200: half A covers local ranks [0, RA)
RB0 = 17 * P                 # 2176: half B covers local ranks [RB0, NLP)
RB_ = NLP - RB0              # 3968 local ranks in half B
NTA = C * RA                 # 25600 rows in tabA
NTB = C * RB_                # 32768 rows in tabB (max idx 32767 = int16 max)
# ranks [RB0, RA) live in BOTH halves; edges whose src falls in that band
# are flex-assigned to whichever region balances per-(core,tile) blocks.
EL = 128              # padded per-view row length (bf16 -> 256B)
EL2 = 2 * EL          # pair row: [view1 | view2] -> 512B in bf16
KS = 4                # k-slabs for the x @ W1 matmul (500 = 4*125)
KSL = FIN // KS       # 125
SC = 5                # tiles per PSUM superchunk
NSC = (T + SC - 1) // SC  # 10
RBCAP = int(os.environ.get("KERNEL_RBCAP", "49"))

# Phase-1 half-groups (groups of <=4 tiles within each A/B half so each
# half completes contiguously and can stage+allgather early)
HALVES = [(TA_T0, TA_T1), (TB_T0, TB_T1)]

F32 = mybir.dt.float32
BF16 = mybir.dt.bfloat16
F8 = mybir.dt.float8e4
I16 = mybir.dt.int16
TB = BF16
AOp = mybir.AluOpType
SINGLE_PACKET = os.environ.get("KERNEL_SP", "0") == "1"
RB_BUFS = int(os.environ.get("KERNEL_RB_BUFS", "3"))

_cache = {}


def _bc_inner(ap, n):
    """Broadcast AP over a new innermost dim of size n (stride 0)."""
    return bass.AP(ap.tensor, ap.offset, list(ap.ap) + [[0, n]])


def _bc_middle(ap2d, n):
    """[128, F] AP -> [128, n, F] with the middle dim broadcast (stride 0)."""
    a = list(ap2d.ap)
    return bass.AP(ap2d.tensor, ap2d.offset, [a[0], [0, n], a[1]])


def _build(rounds, totb, iw16, hasb1, hasb2, sim_mode=False):
    """rounds: list of (region, b0, [(tile, nb), ...]) shared by both layers.
    totb: total global blocks. Block b's S matrix is s_in[b]."""
    nc = bacc.Bacc(None, num_devices=1 if sim_mode else C,
                   target_bir_lowering=False, debug=False,
                   dynamic_dma_scratch_size=32768)

    # first/last global block per SUPERCHUNK in EXECUTION order (leading
    # region's rounds run first). PSUM start/stop granularity is the whole
    # 2KB bank: exactly one start and one stop per (superchunk, view) bank.
    def mk_first_last(first_region):
        first_b = {}
        last_b = {}
        for pass_reg in (first_region, 1 - first_region):
            for (reg, b0, tl) in rounds:
                if reg != pass_reg:
                    continue
                b = b0
                for (t, nb) in tl:
                    sc = t // SC
                    for i in range(nb):
                        if sc not in first_b:
                            first_b[sc] = b
                        last_b[sc] = b
                        b += 1
        return first_b, last_b

    # ---- inputs ----
    # xt: slab-major transposed x (norm_out folded in on host):
    # xt[v, s, k, t*P + p] = (x_view_v[node(t,p)] * no[node]) [s*KSL + k]
    xt = nc.dram_tensor("xt", [2, KS, KSL, T * P], BF16, kind="ExternalInput")
    idx = nc.dram_tensor("idx", [P, iw16], I16, kind="ExternalInput")
    s_in = nc.dram_tensor("s_in", [P, totb * P], BF16, kind="ExternalInput")
    no_in = nc.dram_tensor("no_in", [P, T], F32, kind="ExternalInput")
    ni_in = nc.dram_tensor("ni_in", [P, T], F32, kind="ExternalInput")
    w1_in = nc.dram_tensor("w1_in", [KS, KSL, H], BF16, kind="ExternalInput")
    w2_in = nc.dram_tensor("w2_in", [H, H], BF16, kind="ExternalInput")
    b1_in = nc.dram_tensor("b1_in", [P, H], F32, kind="ExternalInput")
    a1_in = nc.dram_tensor("a1_in", [P, H], F32, kind="ExternalInput")
    b2_in = nc.dram_tensor("b2_in", [P, H], F32, kind="ExternalInput")
    a2_in = nc.dram_tensor("a2_in", [P, H], F32, kind="ExternalInput")
    wv_in = nc.dram_tensor("wv_in", [P, H + 1], F32, kind="ExternalInput")
    out = nc.dram_tensor("out", [2, NLP], F32, kind="ExternalOutput")
    dbg_mode = os.environ.get("KERNEL_DBG", "0") == "1"
    dbg = (nc.dram_tensor("dbg", [2, T, P, H], F32, kind="ExternalOutput")
           if dbg_mode else None)

    with tile.TileContext(nc) as tc:
        with (
            tc.tile_pool(name="cst", bufs=1) as cst,
            tc.tile_pool(name="wrk", bufs=2) as wrk,
            tc.tile_pool(name="ps", bufs=2, space="PSUM") as ps,
            tc.tile_pool(name="dr", bufs=1, space="DRAM") as dr,
        ):
            # ---- resident constants ----
            idxt = cst.tile([P, iw16], I16)
            nc.sync.dma_start(out=idxt[:], in_=idx[:])
            no_t = cst.tile([P, T], F32)
            nc.sync.dma_start(out=no_t[:], in_=no_in[:])
            ni_t = cst.tile([P, T], F32)
            nc.sync.dma_start(out=ni_t[:], in_=ni_in[:])
            w1t = cst.tile([KSL, KS * H], BF16)
            nc.sync.dma_start(
                out=w1t[:].rearrange("p (s h) -> p s h", s=KS),
                in_=w1_in[:].rearrange("s p h -> p s h"),
            )
            w2t = cst.tile([H, H], BF16)
            nc.sync.dma_start(out=w2t[:], in_=w2_in[:])
            b1r = cst.tile([P, H], F32)
            nc.sync.dma_start(out=b1r[:], in_=b1_in[:])
            a1r = cst.tile([P, H], F32)
            nc.sync.dma_start(out=a1r[:], in_=a1_in[:])
            b2r = cst.tile([P, H], F32)
            nc.sync.dma_start(out=b2r[:], in_=b2_in[:])
            a2r = cst.tile([P, H], F32)
            nc.sync.dma_start(out=a2r[:], in_=a2_in[:])
            wvr = cst.tile([P, H + 1], F32)
            nc.sync.dma_start(out=wvr[:], in_=wv_in[:])
            ident = cst.tile([P, P], F32)
            make_identity(nc, ident[:])
            identb = cst.tile([96, 96], BF16)
            nc.vector.tensor_copy(out=identb[:], in_=ident[0:96, 0:96])

            # Pool touch: advance Pool's vector clock past the idx load
            tch = cst.tile([1, 8], I16, name="tch")
            nc.gpsimd.dma_start(out=tch[0:1, 0:4], in_=idxt[0:1, 0:4])

            # ---- DRAM tables (A/B halves, each one Shared collective) ----
            ypairA = dr.tile([NTA, EL2], TB, addr_space="Shared", name="ypairA")
            ypairB = dr.tile([NTB, EL2], TB, addr_space="Shared", name="ypairB")
            gpairA = dr.tile([NTA, EL2], TB, addr_space="Shared", name="gpairA")
            gpairB = dr.tile([NTB, EL2], TB, addr_space="Shared", name="gpairB")
            psl = dr.tile([NLP, EL2], TB, name="psl")
            gsl = dr.tile([NLP, EL2], TB, name="gsl")

            def allgather_half(sl, tab, half):
                r0, r1 = (0, RA) if half == 0 else (RB0, NLP)
                if sim_mode:
                    nc.sync.dma_start(out=tab[0:r1 - r0, :],
                                      in_=sl[r0:r1, :])
                else:
                    nc.gpsimd.collective_compute(
                        "AllGather", AOp.bypass,
                        replica_groups=[list(range(C))],
                        ins=[sl[r0:r1, :].opt()], outs=[tab[:].opt()],
                    )

            def stage_to_table(stage, sl, t0, nt, v):
                """Write stage tiles [t0, t0+nt) (one view's H cols) into the
                local pair-row slab sl."""
                nc.sync.dma_start(
                    out=bass.AP(sl.tensor,
                                sl.offset + t0 * P * EL2 + v * EL,
                                [[EL2, P], [P * EL2, nt], [1, H]]),
                    in_=stage[:, t0 * H:(t0 + nt) * H].rearrange(
                        "p (t h) -> p t h", h=H),
                )

            # ---- phase 1: both views' tables = (x*no) @ W1 ----
            # W1 slab stationary, x streamed up to 4 tiles (512 node-columns)
            # per matmul -> transposed [96, 512] PSUM; PE-transpose back to
            # node-major per tile. Half A (both views) completes first and
            # allgathers while half B computes.
            ystages = [wrk.tile([P, T * H], TB, tag=f"stage{v}", bufs=1,
                                name=f"ystage_{v}")
                       for v in range(2)]
            # pass 1 computes every tile half B needs (tiles 17-48, since
            # the overlap band ranks [RB0, RA) lives in tiles 17-24) so AG1-B
            # can fire early; pass 2 computes the remaining tiles 0-16.
            for half, (ht0, ht1) in ((1, (RB0 // P, T)), (0, (0, RB0 // P))):
                for v in range(2):
                    ystage = ystages[v]
                    t0 = ht0
                    while t0 < ht1:
                        nt4 = min(ht1, t0 + 4) - t0
                        ncol = nt4 * P
                        yT = ps.tile([96, 4 * P], F32, tag="psv", bufs=6,
                                     space="PSUM", name=f"yT_{half}_{v}_{t0}")
                        for s in range(KS):
                            xsl = wrk.tile([KSL, 4 * P], BF16, tag="xtile",
                                           bufs=4, name=f"xsl_{half}_{v}_{t0}_{s}")
                            nc.sync.dma_start(
                                out=xsl[:, :ncol],
                                in_=xt[v, s][:, t0 * P:t0 * P + ncol])
                            nc.tensor.matmul(
                                out=yT[:, :ncol],
                                lhsT=w1t[:, s * H:(s + 1) * H],
                                rhs=xsl[:, :ncol],
                                start=(s == 0), stop=(s == KS - 1),
                            )
                        ysb = wrk.tile([96, 4 * P], BF16, tag="ysb", bufs=3,
                                       name=f"ysb_{half}_{v}_{t0}")
                        nc.vector.tensor_copy(out=ysb[:, :ncol],
                                              in_=yT[:, :ncol])
                        for i in range(nt4):
                            tp = ps.tile([P, H], BF16, tag="tbank", bufs=1,
                                         space="PSUM", name=f"tp1_{v}_{t0}_{i}")
                            nc.tensor.transpose(
                                out=tp[:], in_=ysb[:, i * P:(i + 1) * P],
                                identity=identb[:])
                            nc.vector.tensor_copy(
                                out=ystage[:, (t0 + i) * H:(t0 + i + 1) * H],
                                in_=tp[:])
                        t0 += nt4
                for vv in range(2):
                    stage_to_table(ystages[vv], psl, ht0, ht1 - ht0, vv)
                if half == 1:
                    allgather_half(psl, ypairB, 1)

            def ag1a():
                # reads psl[0:RA]; rows [RB0, RA) staged in pass 1, rest in
                # pass 2 (RAW tracked via range overlap)
                allgather_half(psl, ypairA, 0)

            # ---- SpMM over a pair table; S-matrix accumulate on PE ----
            def spmm_pair(tabA, tabB, layer, br, ar, hasb,
                          first_region=0, pending_cc=None):
                first_b, last_b = mk_first_last(first_region)
                gstages = []
                zbs = []
                if layer == 1:
                    for v in range(2):
                        gstages.append(wrk.tile([P, T * H], TB,
                                                tag=f"gstage{v}", bufs=1,
                                                name=f"gst_{v}"))
                else:
                    for v in range(2):
                        zb = wrk.tile([P, T], F32, tag=f"zb{v}", bufs=1,
                                      name=f"zb_{v}")
                        zbs.append(zb)

                def do_prep(reg, b0, tl):
                    """Gather + S-matrix stream for one round."""
                    ntot = sum(nb for (t, nb) in tl)
                    rb = wrk.tile([P, RBCAP * EL2], TB, tag="rb", bufs=RB_BUFS,
                                  name=f"rb_{layer}_{reg}_{b0}")
                    src = tabA[:] if reg == 0 else tabB[:]
                    off16 = b0 * 8
                    nc.gpsimd.dma_gather(
                        out_ap=rb[:, :ntot * EL2].rearrange(
                            "p (b e) -> p b e", e=EL2),
                        in_ap=src,
                        idxs_ap=idxt[:, off16:off16 + ntot * 8],
                        num_idxs=ntot * P, num_idxs_reg=ntot * P,
                        elem_size=EL2,
                        single_packet=SINGLE_PACKET,
                    )
                    sst = wrk.tile([P, RBCAP * P], TB, tag="sst", bufs=RB_BUFS,
                                   name=f"sst_{layer}_{reg}_{b0}")
                    nc.sync.dma_start(
                        out=sst[:, :ntot * P],
                        in_=s_in[:, b0 * P:(b0 + ntot) * P],
                    )
                    return rb, sst

                def do_matmuls(reg, b0, tl, rb, sst, cur_psv):
                    i = 0
                    b = b0
                    for (t, nb) in tl:
                        sc = t // SC
                        for _ in range(nb):
                            for v in range(2):
                                nc.tensor.matmul(
                                    out=cur_psv[v][:, (t - sc * SC) * H:
                                                   (t - sc * SC + 1) * H],
                                    lhsT=sst[:, i * P:(i + 1) * P],
                                    rhs=rb[:, i * EL2 + v * EL:
                                           i * EL2 + v * EL + H],
                                    start=(b == first_b[sc]),
                                    stop=(b == last_b[sc]),
                                )
                            i += 1
                            b += 1

                def do_postop_sc(sc, cur_psv):
                    t0 = sc * SC
                    nt = min(T, t0 + SC) - t0
                    nh = nt * H
                    for v in range(2):
                        hbuf = wrk.tile([P, SC * H], F32, tag="hbuf", bufs=4,
                                        name=f"hbuf_{layer}_{sc}_{v}")
                        nc.vector.tensor_copy(out=hbuf[:, :nh],
                                              in_=cur_psv[v][:, :nh])
                        h3 = hbuf[:, :nh].rearrange("p (t h) -> p t h", h=H)
                        nc.vector.tensor_tensor(
                            out=h3, in0=h3,
                            in1=_bc_inner(ni_t[:, t0:t0 + nt], H), op=AOp.mult)
                        if hasb:
                            nc.vector.tensor_tensor(
                                out=h3, in0=h3, in1=_bc_middle(br[:], nt),
                                op=AOp.add)
                        pos = wrk.tile([P, SC * H], F32, tag="pos", bufs=4,
                                       name=f"pos_{layer}_{sc}_{v}")
                        nc.vector.tensor_scalar(
                            out=pos[:, :nh], in0=hbuf[:, :nh],
                            scalar1=0.0, scalar2=None, op0=AOp.max)
                        nc.vector.tensor_scalar(
                            out=hbuf[:, :nh], in0=hbuf[:, :nh],
                            scalar1=0.0, scalar2=None, op0=AOp.min)
                        nc.vector.tensor_tensor(
                            out=h3, in0=h3, in1=_bc_middle(ar[:], nt),
                            op=AOp.mult)
                        nc.vector.tensor_tensor(
                            out=hbuf[:, :nh], in0=hbuf[:, :nh],
                            in1=pos[:, :nh], op=AOp.add)
                        if layer == 1:
                            if dbg_mode:
                                for i in range(nt):
                                    nc.sync.dma_start(
                                        out=dbg[v, t0 + i],
                                        in_=hbuf[:, i * H:(i + 1) * H])
                            nc.vector.tensor_tensor(
                                out=h3, in0=h3,
                                in1=_bc_inner(no_t[:, t0:t0 + nt], H),
                                op=AOp.mult)
                            gps = ps.tile([P, SC * H], F32, tag="pgp", bufs=1,
                                          space="PSUM")
                            for i in range(nt):
                                tp = ps.tile([H, P], F32, tag="tbank", bufs=1,
                                             space="PSUM")
                                nc.tensor.transpose(
                                    out=tp[:],
                                    in_=hbuf[:, i * H:(i + 1) * H],
                                    identity=ident[:])
                                gsT = wrk.tile([H, P], BF16, tag="gsT", bufs=3)
                                nc.vector.tensor_copy(out=gsT[:], in_=tp[:])
                                nc.tensor.matmul(
                                    out=gps[:, i * H:(i + 1) * H],
                                    lhsT=gsT[:], rhs=w2t[:],
                                    start=(i == 0), stop=(i == nt - 1))
                            nc.vector.tensor_copy(
                                out=gstages[v][:, t0 * H:(t0 + nt) * H],
                                in_=gps[:, :nh])
                        else:
                            nc.vector.tensor_tensor(
                                out=h3, in0=h3, in1=_bc_middle(wvr[:, 0:H], nt),
                                op=AOp.mult)
                            nc.vector.tensor_reduce(
                                out=zbs[v][:, t0:t0 + nt], in_=h3,
                                axis=mybir.AxisListType.X, op=AOp.add,
                            )

                # drive: region-A rounds run up to 2 superchunks AHEAD of the
                # B rounds, so at each layer start the A rounds (whose table
                # half allgathers early) cover the B collective's latency.
                # A superchunk's postops fire once its B rounds finish. In
                # layer 1, each gpair half allgathers as soon as its tiles'
                # postops are done (A after sc 4, B after sc 9).
                fr, orr = first_region, 1 - first_region
                seq = []
                f_next = o_next = 0
                while o_next < NSC:
                    while f_next < NSC and f_next < o_next + 3:
                        seq.append((f_next, fr))
                        f_next += 1
                    seq.append((o_next, orr))
                    o_next += 1
                psvs = {}
                for (sc, r) in seq:
                    if r == orr and pending_cc is not None:
                        pending_cc()
                        pending_cc = None
                    if sc not in psvs:
                        psvs[sc] = [
                            ps.tile([P, 4 * P], F32, tag="psv", bufs=6,
                                    space="PSUM", name=f"psv{v}_{layer}_{sc}")
                            for v in range(2)
                        ]
                    cur_psv = psvs[sc]
                    for (reg, b0, tl) in rounds:
                        if reg != r or tl[0][0] // SC != sc:
                            continue
                        rb, sst = do_prep(reg, b0, tl)
                        do_matmuls(reg, b0, tl, rb, sst, cur_psv)
                    if r == orr:
                        do_postop_sc(sc, cur_psv)
                        del psvs[sc]
                        if layer == 1 and sc == 4:
                            for v in range(2):
                                stage_to_table(gstages[v], gsl, TA_T0,
                                               TA_T1 - TA_T0, v)
                            allgather_half(gsl, gpairA, 0)
                        if layer == 1 and sc == NSC - 1:
                            for v in range(2):
                                stage_to_table(gstages[v], gsl, TB_T0,
                                               TB_T1 - TB_T0, v)

                if layer != 1:
                    for v in range(2):
                        nc.vector.tensor_scalar(
                            out=zbs[v][:], in0=zbs[v][:],
                            scalar1=wvr[:, H:H + 1], scalar2=None,
                            op0=AOp.add,
                        )
                        nc.sync.dma_start(
                            out=bass.AP(out, v * NLP, [[1, P], [P, T]]),
                            in_=zbs[v][:],
                        )
                return gstages

            gst1 = spmm_pair(ypairA, ypairB, 1, b1r, a1r, hasb1,
                             first_region=1, pending_cc=ag1a)

            def ag2b():
                allgather_half(gsl, gpairB, 1)

            spmm_pair(gpairA, gpairB, 2, b2r, a2r, hasb2,
                      first_region=0, pending_cc=ag2b)

    nc.finalize()
    return nc


def _wrap16(vals):
    """[n] int array -> wrapped [128, n/16] int16 (replicated across 8 Q7 cores)."""
    n = vals.shape[-1]
    assert n % 16 == 0
    w = vals.reshape(n // 16, 16).T.astype(np.int16)
    return np.tile(w, (8, 1))


def _host_plan(src, dst):
    """Builds l-space mapping, per-edge region, dense blocks, rounds."""
    deg_in = np.bincount(dst, minlength=N)

    # snake-balance nodes over the 392 (t-major, c-minor) groups so every
    # (core, tile) has ~equal total in-degree -> equal block counts
    order = np.argsort(-deg_in, kind="stable")
    ranked = np.full((C, T * P), -1, np.int64)
    fill = np.zeros((T, C), np.int64)
    gsizes = np.full((T, C), P, np.int64)
    gsizes[T - 1, :] = NL - (T - 1) * P  # 106 real slots in last tile
    pos = 0
    rnd = 0
    while pos < N:
        active = [(t, c) for t in range(T) for c in range(C)
                  if fill[t, c] < gsizes[t, c]]
        if rnd % 2 == 1:
            active = active[::-1]
        for (t, c) in active:
            if pos >= N:
                break
            node = order[pos]
            ranked[c, t * P + fill[t, c]] = node
            fill[t, c] += 1
            pos += 1
        rnd += 1
    Lmap = np.empty(N, np.int64)   # node -> table row (c-major, rank-minor)
    Cmap = np.empty(N, np.int64)   # node -> owning core
    Rmap = np.empty(N, np.int64)   # node -> local rank
    for c in range(C):
        m = ranked[c] >= 0
        rk_ = np.where(m)[0]
        nodes = ranked[c][m]
        Lmap[nodes] = c * NLP + rk_
        Cmap[nodes] = c
        Rmap[nodes] = rk_

    c_e = Cmap[dst]
    r_e = Rmap[dst]
    t_e = r_e // P
    p_e = r_e % P

    # per-edge region: forced outside the overlap band [RB0, RA); edges whose
    # src rank falls inside it are flex-assigned to balance per-(c,t) counts
    src_c = Cmap[src]
    src_r = Rmap[src]
    forcedA = src_r < RB0
    forcedB = src_r >= RA
    flex = ~(forcedA | forcedB)
    gid = c_e * T + t_e
    nA = np.bincount(gid[forcedA], minlength=C * T)
    nF = np.bincount(gid[flex], minlength=C * T)
    ntot = np.bincount(gid, minlength=C * T)
    addA = np.clip((ntot + 1) // 2 - nA, 0, nF)
    fidx = np.where(flex)[0]
    fg = gid[fidx]
    forder = np.argsort(fg, kind="stable")
    fo = fidx[forder]
    fgo = fg[forder]
    gstarts = np.concatenate(([0], np.cumsum(np.bincount(fgo, minlength=C * T))))[:-1]
    rank_in_g = np.arange(len(fo)) - gstarts[fgo]
    region = np.ones(E, np.int8)
    region[forcedA] = 0
    region[fo[rank_in_g < addA[fgo]]] = 0
    # row index within the half's table: A: c*RA + rank; B: c*RB_ + rank-RB0
    lsrc_half = np.where(region == 0, src_c * RA + src_r,
                         src_c * RB_ + (src_r - RB0))

    # per (c, t, r) counts and shared block counts
    cnt = np.zeros((C, T, 2), np.int64)
    np.add.at(cnt, (c_e, t_e, region), 1)
    nblk = np.maximum(1, (cnt.max(axis=0) + P - 1) // P)  # [T, 2]

    # global block order: superchunk-major, region inner, tile inner
    rounds = []
    binfo = []  # per global block: (tile, region)
    b0_of = {}  # (t, r) -> first global block idx
    gb = 0
    for sc in range(NSC):
        tiles = range(sc * SC, min(T, (sc + 1) * SC))
        for r in range(2):
            run = []
            for t in tiles:
                b0_of[(t, r)] = gb
                run.append((t, int(nblk[t, r])))
                for _ in range(int(nblk[t, r])):
                    binfo.append((t, r))
                gb += int(nblk[t, r])
            # split run into rounds of <= RBCAP blocks
            cur = []
            cb = 0
            curb0 = b0_of[(run[0][0], r)]
            for (t, nb) in run:
                left = nb
                while left > 0:
                    take = min(left, RBCAP - cb)
                    if take == 0:
                        rounds.append((r, curb0, cur))
                        curb0 += cb
                        cur = []
                        cb = 0
                        continue
                    cur.append((t, take))
                    cb += take
                    left -= take
            if cur:
                rounds.append((r, curb0, cur))
    totb = gb

    # place each edge: (global block, row)
    eorder = np.lexsort((r_e, region.astype(np.int64), t_e, c_e))
    # recompute placement per (c,t,r) run using cumulative position
    ce_s = c_e[eorder]
    te_s = t_e[eorder]
    re_s = region[eorder].astype(np.int64)
    key = (ce_s * T + te_s) * 2 + re_s
    kcnt = np.bincount(key, minlength=C * T * 2)
    kstart = np.concatenate(([0], np.cumsum(kcnt)))[:-1]
    pos_in_run = np.arange(E) - kstart[key]
    b0_arr = np.zeros((T, 2), np.int64)
    for t in range(T):
        for r in range(2):
            b0_arr[t, r] = b0_of[(t, r)]
    blk = b0_arr[te_s, re_s] + pos_in_run // P
    row = pos_in_run % P
    loc = lsrc_half[eorder].astype(np.int16)
    pe_s = p_e[eorder]

    # idx array: [totb*128] int16, pad = 0
    iw16 = totb * 8
    idxflat = np.zeros(totb * P, np.int16)
    idxflat[blk * P + row] = loc
    IDX = np.empty((C, P, iw16), np.int16)
    # per-core idx: only this core's edges (others' blocks stay pad)
    for c in range(C):
        m = ce_s == c
        f = np.zeros(totb * P, np.int16)
        f[blk[m] * P + row[m]] = loc[m]
        IDX[c] = _wrap16(f)

    # S matrices, device layout [C, P(row), totb*P]: sst[r, b*P+c] = S[b,r,c]
    S = np.zeros((C, P, totb * P), ml_dtypes.bfloat16)
    one = ml_dtypes.bfloat16(1.0)
    for c in range(C):
        m = ce_s == c
        S[c, row[m], blk[m] * P + pe_s[m]] = one

    return ranked, Lmap, rounds, totb, iw16, IDX, S


def kernel(x, src, dst, perm, W1, b1, a1, W2, b2, a2, Wm, bm):
    x = np.ascontiguousarray(np.asarray(x, np.float32))
    src = np.asarray(src, np.int64)
    dst = np.asarray(dst, np.int64)
    perm = np.asarray(perm, np.int64)
    W1 = np.asarray(W1, np.float32); W2 = np.asarray(W2, np.float32)
    Wm = np.asarray(Wm, np.float32)
    b1 = np.asarray(b1, np.float32); b2 = np.asarray(b2, np.float32)
    a1 = np.asarray(a1, np.float32); a2 = np.asarray(a2, np.float32)
    bm = np.asarray(bm, np.float32)

    deg_out = np.bincount(src, minlength=N)
    deg_in = np.bincount(dst, minlength=N)
    norm_out = np.maximum(deg_out, 1).astype(np.float32) ** -0.5
    norm_in = np.maximum(deg_in, 1).astype(np.float32) ** -0.5

    ranked, Lmap, rounds, totb, iw16, IDX, S = _host_plan(src, dst)

    def tile_pt(vals_pad):
        return np.ascontiguousarray(vals_pad.reshape(C, T, P).transpose(0, 2, 1))

    rmask = ranked >= 0
    rk = np.where(rmask, ranked, 0)
    no_p = np.where(rmask, norm_out[rk], 0).astype(np.float32)
    ni_p = np.where(rmask, norm_in[rk], 0).astype(np.float32)
    NO = tile_pt(no_p); NI = tile_pt(ni_p)

    # norm_out folded into x on the host (both views scale by the table
    # OWNER row's norm_out)
    xp = np.zeros((2, C, NLP, FIN), ml_dtypes.bfloat16)
    for c in range(C):
        m = rmask[c]
        nodes = rk[c][m]
        sc_ = norm_out[nodes][:, None]
        xp[0, c, m] = (x[nodes] * sc_).astype(ml_dtypes.bfloat16)
        xp[1, c, m] = (x[perm[nodes]] * sc_).astype(ml_dtypes.bfloat16)
    # device layout: xt[v, s, k, t*P + p] = x_view_v[node(t,p), s*KSL + k]
    XT = np.ascontiguousarray(
        xp.reshape(2, C, T, P, KS, KSL).transpose(1, 0, 4, 5, 2, 3)
        .reshape(C, 2, KS, KSL, T * P))

    w1s = np.ascontiguousarray(W1.reshape(KS, KSL, H)).astype(ml_dtypes.bfloat16)
    rep = lambda v: np.ascontiguousarray(np.tile(v[None, :], (P, 1)))
    wv = Wm.sum(axis=1)
    wvb = np.concatenate([wv, [bm.sum()]]).astype(np.float32)
    hasb1 = bool(np.any(b1)); hasb2 = bool(np.any(b2))

    rkey = tuple((r, b0, tuple(tl)) for (r, b0, tl) in rounds)
    ck = (hash(rkey), totb, iw16, hasb1, hasb2,
          os.environ.get("KERNEL_DBG", "0"))
    if ck not in _cache:
        _cache[ck] = _build(rounds, totb, iw16, hasb1, hasb2)
    nc = _cache[ck]

    shared = {
        "w1_in": w1s, "w2_in": W2.astype(ml_dtypes.bfloat16),
        "b1_in": rep(b1), "a1_in": rep(a1),
        "b2_in": rep(b2), "a2_in": rep(a2), "wv_in": rep(wvb),
    }
    in_maps = []
    for c in range(C):
        m = dict(shared)
        m.update({
            "xt": XT[c], "idx": IDX[c], "s_in": S[c],
            "no_in": NO[c], "ni_in": NI[c],
        })
        in_maps.append(m)

    trace = os.environ.get("KERNEL_TRACE", "0") == "1"
    res = run_bass_kernel_spmd(nc, in_maps, core_ids=list(range(C)), trace=trace)
    if res.exec_time_ns is not None:
        print(f"HW exec time: {res.exec_time_ns} ns")
        kernel.last_exec_time_ns = res.exec_time_ns

    z = np.empty((2, N), np.float32)
    for c in range(C):
        o = res.results[c]["out"]
        m = rmask[c]
        z[0, ranked[c][m]] = o[0][m]
        z[1, ranked[c][m]] = o[1][m]
    return np.concatenate([z[0], z[1]]).astype(np.float32)



# revision 49
# speedup vs baseline: 1.0082x; 1.0082x over previous
"""DinkNet GNN (2-layer GraphConv encoder, two views) on 8 Trainium2 NeuronCores.

Strategy (self-contained; shapes hardcoded for the nn_DinkNet_dgl problem):
  - Nodes sharded across 8 cores; l(g) = core*6272 + rank. Graph preprocessing
    (degree snake-balance, edge blocks, selection matrices) happens on the
    host; all heavy FP work runs on the NeuronCores.
  - Math folding: (x*no) @ W1 = no * (x@W1); the corrupted view's table is
    computed directly from host-gathered x[perm] rows (no device-side perm
    gather); final (z@Wm+bm).sum(1) = h2 @ Wm.sum(1) + bm.sum().
  - SpMM over 800k edges: per-core edges packed DENSELY into 128-row blocks
    (tile-major); one GPSIMD dma_gather per (5-tile superchunk, region)
    fetches the src pair-rows (512B each, both views interleaved); a per-block
    one-hot selection matrix S (streamed bf16 from HBM via HWDGE) routes and
    segment-sums rows into the dst tile's PSUM region on the TensorEngine.
    Dense packing avoids the per-(dst,slot) padding of slot-structured
    gathers; desc-gen on the Q7 cores is the bottleneck, so fewer gather
    descriptors = faster.
  - The table is split into two Shared tensors: A = all cores' local tiles
    [0,25) and B = tiles [25,49). Each half allgathers with a single
    collective as soon as that half is produced (A at mid-phase), so the
    collectives overlap compute/desc-gen and region-A gather rounds can
    start before the B collective lands. Both halves stay within int16
    gather addressing (<= 25600 rows).
"""
import os
import numpy as np
import ml_dtypes

import concourse.bass as bass
import concourse.bacc as bacc
import concourse.mybir as mybir
import concourse.tile as tile
from concourse.bass_utils import run_bass_kernel_spmd
from concourse.masks import make_identity

# Problem shapes (hardcoded per contract).
N, E, FIN, H = 50000, 800000, 500, 96
C = 8                 # cores
P = 128               # partitions
NL = N // C           # 6250 real nodes per core
T = (NL + P - 1) // P # 49 rank tiles per core
NLP = T * P           # 6272 padded nodes per core
NTAB = C * NLP        # 50176 table rows (conceptually)
# A/B table split: half A = local tiles [0, 25) (ranks [0, 3200)), half B =
# tiles [25, 49) (ranks [3200, 6272)). Each half is its own Shared tensor,
# allgathered by a single collective as soon as that half is produced; both
# halves stay within int16 gather addressing (< 32768 rows).
TA_T0, TA_T1 = 0, 25         # tiles in half A
TB_T0, TB_T1 = 25, T         # tiles computed in phase-1's B pass
RA = TA_T1 * P               # 3200: half A covers local ranks [0, RA)
RB0 = 17 * P                 # 2176: half B covers local ranks [RB0, NLP)
RB_ = NLP - RB0              # 3968 local ranks in half B
NTA = C * RA                 # 25600 rows in tabA
NTB = C * RB_                # 32768 rows in tabB (max idx 32767 = int16 max)
# ranks [RB0, RA) live in BOTH halves; edges whose src falls in that band
# are flex-assigned to whichever region balances per-(core,tile) blocks.
EL = 128              # padded per-view row length (bf16 -> 256B)
EL2 = 2 * EL          # pair row: [view1 | view2] -> 512B in bf16
KS = 4                # k-slabs for the x @ W1 matmul (500 = 4*125)
KSL = FIN // KS       # 125
SC = 5                # tiles per PSUM superchunk
NSC = (T + SC - 1) // SC  # 10
RBCAP = int(os.environ.get("KERNEL_RBCAP", "49"))

# Phase-1 half-groups (groups of <=4 tiles within each A/B half so each
# half completes contiguously and can stage+allgather early)
HALVES = [(TA_T0, TA_T1), (TB_T0, TB_T1)]

F32 = mybir.dt.float32
BF16 = mybir.dt.bfloat16
F8 = mybir.dt.float8e4
I16 = mybir.dt.int16
TB = BF16
AOp = mybir.AluOpType
SINGLE_PACKET = os.environ.get("KERNEL_SP", "0") == "1"
RB_BUFS = int(os.environ.get("KERNEL_RB_BUFS", "3"))

_cache = {}


def _bc_inner(ap, n):
    """Broadcast AP over a new innermost dim of size n (stride 0)."""
    return bass.AP(ap.tensor, ap.offset, list(ap.ap) + [[0, n]])


def _bc_middle(ap2d, n):
    """[128, F] AP -> [128, n, F] with the middle dim broadcast (stride 0)."""
    a = list(ap2d.ap)
    return bass.AP(ap2d.tensor, ap2d.offset, [a[0], [0, n], a[1]])


def _build(rounds, totb, iw16, hasb1, hasb2, sim_mode=False):
    """rounds: list of (region, b0, [(tile, nb), ...]) shared by both layers.
    totb: total global blocks. Block b's S matrix is s_in[b]."""
    nc = bacc.Bacc(None, num_devices=1 if sim_mode else C,
                   target_bir_lowering=False, debug=False,
                   dynamic_dma_scratch_size=32768)

    # first/last global block per SUPERCHUNK in EXECUTION order (leading
    # region's rounds run first). PSUM start/stop granularity is the whole
    # 2KB bank: exactly one start and one stop per (superchunk, view) bank.
    def mk_first_last(first_region):
        first_b = {}
        last_b = {}
        for pass_reg in (first_region, 1 - first_region):
            for (reg, b0, tl) in rounds:
                if reg != pass_reg:
                    continue
                b = b0
                for (t, nb) in tl:
                    sc = t // SC
                    for i in range(nb):
                        if sc not in first_b:
                            first_b[sc] = b
                        last_b[sc] = b
                        b += 1
        return first_b, last_b

    # ---- inputs ----
    # xt: slab-major transposed x (norm_out folded in on host):
    # xt[v, s, k, t*P + p] = (x_view_v[node(t,p)] * no[node]) [s*KSL + k]
    xt = nc.dram_tensor("xt", [2, KS, KSL, T * P], BF16, kind="ExternalInput")
    idx = nc.dram_tensor("idx", [P, iw16], I16, kind="ExternalInput")
    s_in = nc.dram_tensor("s_in", [P, totb * P], BF16, kind="ExternalInput")
    no_in = nc.dram_tensor("no_in", [P, T], F32, kind="ExternalInput")
    ni_in = nc.dram_tensor("ni_in", [P, T], F32, kind="ExternalInput")
    w1_in = nc.dram_tensor("w1_in", [KS, KSL, H], BF16, kind="ExternalInput")
    w2_in = nc.dram_tensor("w2_in", [H, H], BF16, kind="ExternalInput")
    b1_in = nc.dram_tensor("b1_in", [P, H], F32, kind="ExternalInput")
    a1_in = nc.dram_tensor("a1_in", [P, H], F32, kind="ExternalInput")
    b2_in = nc.dram_tensor("b2_in", [P, H], F32, kind="ExternalInput")
    a2_in = nc.dram_tensor("a2_in", [P, H], F32, kind="ExternalInput")
    wv_in = nc.dram_tensor("wv_in", [P, H + 1], F32, kind="ExternalInput")
    out = nc.dram_tensor("out", [2, NLP], F32, kind="ExternalOutput")
    dbg_mode = os.environ.get("KERNEL_DBG", "0") == "1"
    dbg = (nc.dram_tensor("dbg", [2, T, P, H], F32, kind="ExternalOutput")
           if dbg_mode else None)

    with tile.TileContext(nc) as tc:
        with (
            tc.tile_pool(name="cst", bufs=1) as cst,
            tc.tile_pool(name="wrk", bufs=2) as wrk,
            tc.tile_pool(name="ps", bufs=2, space="PSUM") as ps,
            tc.tile_pool(name="dr", bufs=1, space="DRAM") as dr,
        ):
            # ---- resident constants ----
            idxt = cst.tile([P, iw16], I16)
            nc.sync.dma_start(out=idxt[:], in_=idx[:])
            no_t = cst.tile([P, T], F32)
            nc.sync.dma_start(out=no_t[:], in_=no_in[:])
            ni_t = cst.tile([P, T], F32)
            nc.sync.dma_start(out=ni_t[:], in_=ni_in[:])
            w1t = cst.tile([KSL, KS * H], BF16)
            nc.sync.dma_start(
                out=w1t[:].rearrange("p (s h) -> p s h", s=KS),
                in_=w1_in[:].rearrange("s p h -> p s h"),
            )
            w2t = cst.tile([H, H], BF16)
            nc.sync.dma_start(out=w2t[:], in_=w2_in[:])
            b1r = cst.tile([P, H], F32)
            nc.sync.dma_start(out=b1r[:], in_=b1_in[:])
            a1r = cst.tile([P, H], F32)
            nc.sync.dma_start(out=a1r[:], in_=a1_in[:])
            b2r = cst.tile([P, H], F32)
            nc.sync.dma_start(out=b2r[:], in_=b2_in[:])
            a2r = cst.tile([P, H], F32)
            nc.sync.dma_start(out=a2r[:], in_=a2_in[:])
            wvr = cst.tile([P, H + 1], F32)
            nc.sync.dma_start(out=wvr[:], in_=wv_in[:])
            ident = cst.tile([P, P], F32)
            make_identity(nc, ident[:])
            identb = cst.tile([96, 96], BF16)
            nc.vector.tensor_copy(out=identb[:], in_=ident[0:96, 0:96])

            # Pool touch: advance Pool's vector clock past the idx load
            tch = cst.tile([1, 8], I16, name="tch")
            nc.gpsimd.dma_start(out=tch[0:1, 0:4], in_=idxt[0:1, 0:4])

            # ---- DRAM tables (A/B halves, each one Shared collective) ----
            ypairA = dr.tile([NTA, EL2], TB, addr_space="Shared", name="ypairA")
            ypairB = dr.tile([NTB, EL2], TB, addr_space="Shared", name="ypairB")
            gpairA = dr.tile([NTA, EL2], TB, addr_space="Shared", name="gpairA")
            gpairB = dr.tile([NTB, EL2], TB, addr_space="Shared", name="gpairB")
            psl = dr.tile([NLP, EL2], TB, name="psl")
            gsl = dr.tile([NLP, EL2], TB, name="gsl")

            def allgather_half(sl, tab, half):
                r0, r1 = (0, RA) if half == 0 else (RB0, NLP)
                if sim_mode:
                    nc.sync.dma_start(out=tab[0:r1 - r0, :],
                                      in_=sl[r0:r1, :])
                else:
                    nc.gpsimd.collective_compute(
                        "AllGather", AOp.bypass,
                        replica_groups=[list(range(C))],
                        ins=[sl[r0:r1, :].opt()], outs=[tab[:].opt()],
                    )

            def stage_to_table(stage, sl, t0, nt, v):
                """Write stage tiles [t0, t0+nt) (one view's H cols) into the
                local pair-row slab sl."""
                nc.sync.dma_start(
                    out=bass.AP(sl.tensor,
                                sl.offset + t0 * P * EL2 + v * EL,
                                [[EL2, P], [P * EL2, nt], [1, H]]),
                    in_=stage[:, t0 * H:(t0 + nt) * H].rearrange(
                        "p (t h) -> p t h", h=H),
                )

            # ---- phase 1: both views' tables = (x*no) @ W1 ----
            # W1 slab stationary, x streamed up to 4 tiles (512 node-columns)
            # per matmul -> transposed [96, 512] PSUM; PE-transpose back to
            # node-major per tile. Half A (both views) completes first and
            # allgathers while half B computes.
            ystages = [wrk.tile([P, T * H], TB, tag=f"stage{v}", bufs=1,
                                name=f"ystage_{v}")
                       for v in range(2)]
            for half, (ht0, ht1) in ((0, HALVES[0]), (1, HALVES[1])):
                for v in range(2):
                    ystage = ystages[v]
                    t0 = ht0
                    while t0 < ht1:
                        nt4 = min(ht1, t0 + 4) - t0
                        ncol = nt4 * P
                        yT = ps.tile([96, 4 * P], F32, tag="psv", bufs=6,
                                     space="PSUM", name=f"yT_{half}_{v}_{t0}")
                        for s in range(KS):
                            xsl = wrk.tile([KSL, 4 * P], BF16, tag="xtile",
                                           bufs=4, name=f"xsl_{half}_{v}_{t0}_{s}")
                            nc.sync.dma_start(
                                out=xsl[:, :ncol],
                                in_=xt[v, s][:, t0 * P:t0 * P + ncol])
                            nc.tensor.matmul(
                                out=yT[:, :ncol],
                                lhsT=w1t[:, s * H:(s + 1) * H],
                                rhs=xsl[:, :ncol],
                                start=(s == 0), stop=(s == KS - 1),
                            )
                        ysb = wrk.tile([96, 4 * P], BF16, tag="ysb", bufs=3,
                                       name=f"ysb_{half}_{v}_{t0}")
                        nc.vector.tensor_copy(out=ysb[:, :ncol],
                                              in_=yT[:, :ncol])
                        for i in range(nt4):
                            tp = ps.tile([P, H], BF16, tag="tbank", bufs=1,
                                         space="PSUM", name=f"tp1_{v}_{t0}_{i}")
                            nc.tensor.transpose(
                                out=tp[:], in_=ysb[:, i * P:(i + 1) * P],
                                identity=identb[:])
                            nc.vector.tensor_copy(
                                out=ystage[:, (t0 + i) * H:(t0 + i + 1) * H],
                                in_=tp[:])
                        t0 += nt4
                for vv in range(2):
                    stage_to_table(ystages[vv], psl, ht0, ht1 - ht0, vv)
                if half == 0:
                    allgather_half(psl, ypairA, 0)

            def ag1b():
                allgather_half(psl, ypairB, 1)

            # ---- SpMM over a pair table; S-matrix accumulate on PE ----
            def spmm_pair(tabA, tabB, layer, br, ar, hasb,
                          first_region=0, pending_cc=None):
                first_b, last_b = mk_first_last(first_region)
                gstages = []
                zbs = []
                if layer == 1:
                    for v in range(2):
                        gstages.append(wrk.tile([P, T * H], TB,
                                                tag=f"gstage{v}", bufs=1,
                                                name=f"gst_{v}"))
                else:
                    for v in range(2):
                        zb = wrk.tile([P, T], F32, tag=f"zb{v}", bufs=1,
                                      name=f"zb_{v}")
                        zbs.append(zb)

                def do_prep(reg, b0, tl):
                    """Gather + S-matrix stream for one round."""
                    ntot = sum(nb for (t, nb) in tl)
                    rb = wrk.tile([P, RBCAP * EL2], TB, tag="rb", bufs=RB_BUFS,
                                  name=f"rb_{layer}_{reg}_{b0}")
                    src = tabA[:] if reg == 0 else tabB[:]
                    off16 = b0 * 8
                    nc.gpsimd.dma_gather(
                        out_ap=rb[:, :ntot * EL2].rearrange(
                            "p (b e) -> p b e", e=EL2),
                        in_ap=src,
                        idxs_ap=idxt[:, off16:off16 + ntot * 8],
                        num_idxs=ntot * P, num_idxs_reg=ntot * P,
                        elem_size=EL2,
                        single_packet=SINGLE_PACKET,
                    )
                    sst = wrk.tile([P, RBCAP * P], TB, tag="sst", bufs=RB_BUFS,
                                   name=f"sst_{layer}_{reg}_{b0}")
                    nc.sync.dma_start(
                        out=sst[:, :ntot * P],
                        in_=s_in[:, b0 * P:(b0 + ntot) * P],
                    )
                    return rb, sst

                def do_matmuls(reg, b0, tl, rb, sst, cur_psv):
                    i = 0
                    b = b0
                    for (t, nb) in tl:
                        sc = t // SC
                        for _ in range(nb):
                            for v in range(2):
                                nc.tensor.matmul(
                                    out=cur_psv[v][:, (t - sc * SC) * H:
                                                   (t - sc * SC + 1) * H],
                                    lhsT=sst[:, i * P:(i + 1) * P],
                                    rhs=rb[:, i * EL2 + v * EL:
                                           i * EL2 + v * EL + H],
                                    start=(b == first_b[sc]),
                                    stop=(b == last_b[sc]),
                                )
                            i += 1
                            b += 1

                def do_postop_sc(sc, cur_psv):
                    t0 = sc * SC
                    nt = min(T, t0 + SC) - t0
                    nh = nt * H
                    for v in range(2):
                        hbuf = wrk.tile([P, SC * H], F32, tag="hbuf", bufs=4,
                                        name=f"hbuf_{layer}_{sc}_{v}")
                        nc.vector.tensor_copy(out=hbuf[:, :nh],
                                              in_=cur_psv[v][:, :nh])
                        h3 = hbuf[:, :nh].rearrange("p (t h) -> p t h", h=H)
                        nc.vector.tensor_tensor(
                            out=h3, in0=h3,
                            in1=_bc_inner(ni_t[:, t0:t0 + nt], H), op=AOp.mult)
                        if hasb:
                            nc.vector.tensor_tensor(
                                out=h3, in0=h3, in1=_bc_middle(br[:], nt),
                                op=AOp.add)
                        pos = wrk.tile([P, SC * H], F32, tag="pos", bufs=4,
                                       name=f"pos_{layer}_{sc}_{v}")
                        nc.vector.tensor_scalar(
                            out=pos[:, :nh], in0=hbuf[:, :nh],
                            scalar1=0.0, scalar2=None, op0=AOp.max)
                        nc.vector.tensor_scalar(
                            out=hbuf[:, :nh], in0=hbuf[:, :nh],
                            scalar1=0.0, scalar2=None, op0=AOp.min)
                        nc.vector.tensor_tensor(
                            out=h3, in0=h3, in1=_bc_middle(ar[:], nt),
                            op=AOp.mult)
                        nc.vector.tensor_tensor(
                            out=hbuf[:, :nh], in0=hbuf[:, :nh],
                            in1=pos[:, :nh], op=AOp.add)
                        if layer == 1:
                            if dbg_mode:
                                for i in range(nt):
                                    nc.sync.dma_start(
                                        out=dbg[v, t0 + i],
                                        in_=hbuf[:, i * H:(i + 1) * H])
                            nc.vector.tensor_tensor(
                                out=h3, in0=h3,
                                in1=_bc_inner(no_t[:, t0:t0 + nt], H),
                                op=AOp.mult)
                            gps = ps.tile([P, SC * H], F32, tag="pgp", bufs=1,
                                          space="PSUM")
                            for i in range(nt):
                                tp = ps.tile([H, P], F32, tag="tbank", bufs=1,
                                             space="PSUM")
                                nc.tensor.transpose(
                                    out=tp[:],
                                    in_=hbuf[:, i * H:(i + 1) * H],
                                    identity=ident[:])
                                gsT = wrk.tile([H, P], BF16, tag="gsT", bufs=3)
                                nc.vector.tensor_copy(out=gsT[:], in_=tp[:])
                                nc.tensor.matmul(
                                    out=gps[:, i * H:(i + 1) * H],
                                    lhsT=gsT[:], rhs=w2t[:],
                                    start=(i == 0), stop=(i == nt - 1))
                            nc.vector.tensor_copy(
                                out=gstages[v][:, t0 * H:(t0 + nt) * H],
                                in_=gps[:, :nh])
                        else:
                            nc.vector.tensor_tensor(
                                out=h3, in0=h3, in1=_bc_middle(wvr[:, 0:H], nt),
                                op=AOp.mult)
                            nc.vector.tensor_reduce(
                                out=zbs[v][:, t0:t0 + nt], in_=h3,
                                axis=mybir.AxisListType.X, op=AOp.add,
                            )

                # drive: region-A rounds run up to 2 superchunks AHEAD of the
                # B rounds, so at each layer start the A rounds (whose table
                # half allgathers early) cover the B collective's latency.
                # A superchunk's postops fire once its B rounds finish. In
                # layer 1, each gpair half allgathers as soon as its tiles'
                # postops are done (A after sc 4, B after sc 9).
                fr, orr = first_region, 1 - first_region
                seq = []
                f_next = o_next = 0
                while o_next < NSC:
                    while f_next < NSC and f_next < o_next + 3:
                        seq.append((f_next, fr))
                        f_next += 1
                    seq.append((o_next, orr))
                    o_next += 1
                psvs = {}
                for (sc, r) in seq:
                    if r == orr and pending_cc is not None:
                        pending_cc()
                        pending_cc = None
                    if sc not in psvs:
                        psvs[sc] = [
                            ps.tile([P, 4 * P], F32, tag="psv", bufs=6,
                                    space="PSUM", name=f"psv{v}_{layer}_{sc}")
                            for v in range(2)
                        ]
                    cur_psv = psvs[sc]
                    for (reg, b0, tl) in rounds:
                        if reg != r or tl[0][0] // SC != sc:
                            continue
                        rb, sst = do_prep(reg, b0, tl)
                        do_matmuls(reg, b0, tl, rb, sst, cur_psv)
                    if r == orr:
                        do_postop_sc(sc, cur_psv)
                        del psvs[sc]
                        if layer == 1 and sc == 4:
                            for v in range(2):
                                stage_to_table(gstages[v], gsl, TA_T0,
                                               TA_T1 - TA_T0, v)
                            allgather_half(gsl, gpairA, 0)
                        if layer == 1 and sc == NSC - 1:
                            for v in range(2):
                                stage_to_table(gstages[v], gsl, TB_T0,
                                               TB_T1 - TB_T0, v)

                if layer != 1:
                    for v in range(2):
                        nc.vector.tensor_scalar(
                            out=zbs[v][:], in0=zbs[v][:],
                            scalar1=wvr[:, H:H + 1], scalar2=None,
                            op0=AOp.add,
                        )
                        nc.sync.dma_start(
                            out=bass.AP(out, v * NLP, [[1, P], [P, T]]),
                            in_=zbs[v][:],
                        )
                return gstages

            gst1 = spmm_pair(ypairA, ypairB, 1, b1r, a1r, hasb1,
                             first_region=0, pending_cc=ag1b)

            def ag2b():
                allgather_half(gsl, gpairB, 1)

            spmm_pair(gpairA, gpairB, 2, b2r, a2r, hasb2,
                      first_region=0, pending_cc=ag2b)

    nc.finalize()
    return nc


def _wrap16(vals):
    """[n] int array -> wrapped [128, n/16] int16 (replicated across 8 Q7 cores)."""
    n = vals.shape[-1]
    assert n % 16 == 0
    w = vals.reshape(n // 16, 16).T.astype(np.int16)
    return np.tile(w, (8, 1))


def _host_plan(src, dst):
    """Builds l-space mapping, per-edge region, dense blocks, rounds."""
    deg_in = np.bincount(dst, minlength=N)

    # snake-balance nodes over the 392 (t-major, c-minor) groups so every
    # (core, tile) has ~equal total in-degree -> equal block counts
    order = np.argsort(-deg_in, kind="stable")
    ranked = np.full((C, T * P), -1, np.int64)
    fill = np.zeros((T, C), np.int64)
    gsizes = np.full((T, C), P, np.int64)
    gsizes[T - 1, :] = NL - (T - 1) * P  # 106 real slots in last tile
    pos = 0
    rnd = 0
    while pos < N:
        active = [(t, c) for t in range(T) for c in range(C)
                  if fill[t, c] < gsizes[t, c]]
        if rnd % 2 == 1:
            active = active[::-1]
        for (t, c) in active:
            if pos >= N:
                break
            node = order[pos]
            ranked[c, t * P + fill[t, c]] = node
            fill[t, c] += 1
            pos += 1
        rnd += 1
    Lmap = np.empty(N, np.int64)   # node -> table row (c-major, rank-minor)
    Cmap = np.empty(N, np.int64)   # node -> owning core
    Rmap = np.empty(N, np.int64)   # node -> local rank
    for c in range(C):
        m = ranked[c] >= 0
        rk_ = np.where(m)[0]
        nodes = ranked[c][m]
        Lmap[nodes] = c * NLP + rk_
        Cmap[nodes] = c
        Rmap[nodes] = rk_

    c_e = Cmap[dst]
    r_e = Rmap[dst]
    t_e = r_e // P
    p_e = r_e % P

    # per-edge region: forced outside the overlap band [RB0, RA); edges whose
    # src rank falls inside it are flex-assigned to balance per-(c,t) counts
    src_c = Cmap[src]
    src_r = Rmap[src]
    forcedA = src_r < RB0
    forcedB = src_r >= RA
    flex = ~(forcedA | forcedB)
    gid = c_e * T + t_e
    nA = np.bincount(gid[forcedA], minlength=C * T)
    nF = np.bincount(gid[flex], minlength=C * T)
    ntot = np.bincount(gid, minlength=C * T)
    addA = np.clip((ntot + 1) // 2 - nA, 0, nF)
    fidx = np.where(flex)[0]
    fg = gid[fidx]
    forder = np.argsort(fg, kind="stable")
    fo = fidx[forder]
    fgo = fg[forder]
    gstarts = np.concatenate(([0], np.cumsum(np.bincount(fgo, minlength=C * T))))[:-1]
    rank_in_g = np.arange(len(fo)) - gstarts[fgo]
    region = np.ones(E, np.int8)
    region[forcedA] = 0
    region[fo[rank_in_g < addA[fgo]]] = 0
    # row index within the half's table: A: c*RA + rank; B: c*RB_ + rank-RB0
    lsrc_half = np.where(region == 0, src_c * RA + src_r,
                         src_c * RB_ + (src_r - RB0))

    # per (c, t, r) counts and shared block counts
    cnt = np.zeros((C, T, 2), np.int64)
    np.add.at(cnt, (c_e, t_e, region), 1)
    nblk = np.maximum(1, (cnt.max(axis=0) + P - 1) // P)  # [T, 2]

    # global block order: superchunk-major, region inner, tile inner
    rounds = []
    binfo = []  # per global block: (tile, region)
    b0_of = {}  # (t, r) -> first global block idx
    gb = 0
    for sc in range(NSC):
        tiles = range(sc * SC, min(T, (sc + 1) * SC))
        for r in range(2):
            run = []
            for t in tiles:
                b0_of[(t, r)] = gb
                run.append((t, int(nblk[t, r])))
                for _ in range(int(nblk[t, r])):
                    binfo.append((t, r))
                gb += int(nblk[t, r])
            # split run into rounds of <= RBCAP blocks
            cur = []
            cb = 0
            curb0 = b0_of[(run[0][0], r)]
            for (t, nb) in run:
                left = nb
                while left > 0:
                    take = min(left, RBCAP - cb)
                    if take == 0:
                        rounds.append((r, curb0, cur))
                        curb0 += cb
                        cur = []
                        cb = 0
                        continue
                    cur.append((t, take))
                    cb += take
                    left -= take
            if cur:
                rounds.append((r, curb0, cur))
    totb = gb

    # place each edge: (global block, row)
    eorder = np.lexsort((r_e, region.astype(np.int64), t_e, c_e))
    # recompute placement per (c,t,r) run using cumulative position
    ce_s = c_e[eorder]
    te_s = t_e[eorder]
    re_s = region[eorder].astype(np.int64)
    key = (ce_s * T + te_s) * 2 + re_s
    kcnt = np.bincount(key, minlength=C * T * 2)
    kstart = np.concatenate(([0], np.cumsum(kcnt)))[:-1]
    pos_in_run = np.arange(E) - kstart[key]
    b0_arr = np.zeros((T, 2), np.int64)
    for t in range(T):
        for r in range(2):
            b0_arr[t, r] = b0_of[(t, r)]
    blk = b0_arr[te_s, re_s] + pos_in_run // P
    row = pos_in_run % P
    loc = lsrc_half[eorder].astype(np.int16)
    pe_s = p_e[eorder]

    # idx array: [totb*128] int16, pad = 0
    iw16 = totb * 8
    idxflat = np.zeros(totb * P, np.int16)
    idxflat[blk * P + row] = loc
    IDX = np.empty((C, P, iw16), np.int16)
    # per-core idx: only this core's edges (others' blocks stay pad)
    for c in range(C):
        m = ce_s == c
        f = np.zeros(totb * P, np.int16)
        f[blk[m] * P + row[m]] = loc[m]
        IDX[c] = _wrap16(f)

    # S matrices, device layout [C, P(row), totb*P]: sst[r, b*P+c] = S[b,r,c]
    S = np.zeros((C, P, totb * P), ml_dtypes.bfloat16)
    one = ml_dtypes.bfloat16(1.0)
    for c in range(C):
        m = ce_s == c
        S[c, row[m], blk[m] * P + pe_s[m]] = one

    return ranked, Lmap, rounds, totb, iw16, IDX, S


def kernel(x, src, dst, perm, W1, b1, a1, W2, b2, a2, Wm, bm):
    x = np.ascontiguousarray(np.asarray(x, np.float32))
    src = np.asarray(src, np.int64)
    dst = np.asarray(dst, np.int64)
    perm = np.asarray(perm, np.int64)
    W1 = np.asarray(W1, np.float32); W2 = np.asarray(W2, np.float32)
    Wm = np.asarray(Wm, np.float32)
    b1 = np.asarray(b1, np.float32); b2 = np.asarray(b2, np.float32)
    a1 = np.asarray(a1, np.float32); a2 = np.asarray(a2, np.float32)
    bm = np.asarray(bm, np.float32)

    deg_out = np.bincount(src, minlength=N)
    deg_in = np.bincount(dst, minlength=N)
    norm_out = np.maximum(deg_out, 1).astype(np.float32) ** -0.5
    norm_in = np.maximum(deg_in, 1).astype(np.float32) ** -0.5

    ranked, Lmap, rounds, totb, iw16, IDX, S = _host_plan(src, dst)

    def tile_pt(vals_pad):
        return np.ascontiguousarray(vals_pad.reshape(C, T, P).transpose(0, 2, 1))

    rmask = ranked >= 0
    rk = np.where(rmask, ranked, 0)
    no_p = np.where(rmask, norm_out[rk], 0).astype(np.float32)
    ni_p = np.where(rmask, norm_in[rk], 0).astype(np.float32)
    NO = tile_pt(no_p); NI = tile_pt(ni_p)

    # norm_out folded into x on the host (both views scale by the table
    # OWNER row's norm_out)
    xp = np.zeros((2, C, NLP, FIN), ml_dtypes.bfloat16)
    for c in range(C):
        m = rmask[c]
        nodes = rk[c][m]
        sc_ = norm_out[nodes][:, None]
        xp[0, c, m] = (x[nodes] * sc_).astype(ml_dtypes.bfloat16)
        xp[1, c, m] = (x[perm[nodes]] * sc_).astype(ml_dtypes.bfloat16)
    # device layout: xt[v, s, k, t*P + p] = x_view_v[node(t,p), s*KSL + k]
    XT = np.ascontiguousarray(
        xp.reshape(2, C, T, P, KS, KSL).transpose(1, 0, 4, 5, 2, 3)
        .reshape(C, 2, KS, KSL, T * P))

    w1s = np.ascontiguousarray(W1.reshape(KS, KSL, H)).astype(ml_dtypes.bfloat16)
    rep = lambda v: np.ascontiguousarray(np.tile(v[None, :], (P, 1)))
    wv = Wm.sum(axis=1)
    wvb = np.concatenate([wv, [bm.sum()]]).astype(np.float32)
    hasb1 = bool(np.any(b1)); hasb2 = bool(np.any(b2))

    rkey = tuple((r, b0, tuple(tl)) for (r, b0, tl) in rounds)
    ck = (hash(rkey), totb, iw16, hasb1, hasb2,
          os.environ.get("KERNEL_DBG", "0"))
    if ck not in _cache:
        _cache[ck] = _build(rounds, totb, iw16, hasb1, hasb2)
    nc = _cache[ck]

    shared = {
        "w1_in": w1s, "w2_in": W2.astype(ml_dtypes.bfloat16),
        "b1_in": rep(b1), "a1_in": rep(a1),
        "b2_in": rep(b2), "a2_in": rep(a2), "wv_in": rep(wvb),
    }
    in_maps = []
    for c in range(C):
        m = dict(shared)
        m.update({
            "xt": XT[c], "idx": IDX[c], "s_in": S[c],
            "no_in": NO[c], "ni_in": NI[c],
        })
        in_maps.append(m)

    trace = os.environ.get("KERNEL_TRACE", "0") == "1"
    res = run_bass_kernel_spmd(nc, in_maps, core_ids=list(range(C)), trace=trace)
    if res.exec_time_ns is not None:
        print(f"HW exec time: {res.exec_time_ns} ns")
        kernel.last_exec_time_ns = res.exec_time_ns

    z = np.empty((2, N), np.float32)
    for c in range(C):
        o = res.results[c]["out"]
        m = rmask[c]
        z[0, ranked[c][m]] = o[0][m]
        z[1, ranked[c][m]] = o[1][m]
    return np.concatenate([z[0], z[1]]).astype(np.float32)



# revision 51
# speedup vs baseline: 1.0109x; 1.0027x over previous
"""DinkNet GNN (2-layer GraphConv encoder, two views) on 8 Trainium2 NeuronCores.

Strategy (self-contained; shapes hardcoded for the nn_DinkNet_dgl problem):
  - Nodes sharded across 8 cores; l(g) = core*6272 + rank. Graph preprocessing
    (degree snake-balance, edge blocks, selection matrices) happens on the
    host; all heavy FP work runs on the NeuronCores.
  - Math folding: (x*no) @ W1 = no * (x@W1); the corrupted view's table is
    computed directly from host-gathered x[perm] rows (no device-side perm
    gather); final (z@Wm+bm).sum(1) = h2 @ Wm.sum(1) + bm.sum().
  - SpMM over 800k edges: per-core edges packed DENSELY into 128-row blocks
    (tile-major); one GPSIMD dma_gather per (5-tile superchunk, region)
    fetches the src pair-rows (512B each, both views interleaved); a per-block
    one-hot selection matrix S (streamed bf16 from HBM via HWDGE) routes and
    segment-sums rows into the dst tile's PSUM region on the TensorEngine.
    Dense packing avoids the per-(dst,slot) padding of slot-structured
    gathers; desc-gen on the Q7 cores is the bottleneck, so fewer gather
    descriptors = faster.
  - The table is split into two Shared tensors: A = all cores' local ranks
    [0, 3200) and B = ranks [2176, 6272) (overlapping flex band so edges can
    be region-balanced). Each half allgathers with one collective as soon as
    it is produced (A at mid-phase-1 / after L1 superchunk 4), so collectives
    overlap compute/desc-gen; each layer's round sequence runs region-A
    rounds up to 3 superchunks ahead so they cover the B collective's
    latency. Both halves stay within int16 gather addressing (<= 32768 rows).
  - Phase 1 streams x through W1-stationary 512-column matmuls (transposed
    [96, cols] PSUM) and PE-transposes per tile back to node-major; norm_out
    is folded into x on the host.
"""
import os
import numpy as np
import ml_dtypes

import concourse.bass as bass
import concourse.bacc as bacc
import concourse.mybir as mybir
import concourse.tile as tile
from concourse.bass_utils import run_bass_kernel_spmd
from concourse.masks import make_identity

# Problem shapes (hardcoded per contract).
N, E, FIN, H = 50000, 800000, 500, 96
C = 8                 # cores
P = 128               # partitions
NL = N // C           # 6250 real nodes per core
T = (NL + P - 1) // P # 49 rank tiles per core
NLP = T * P           # 6272 padded nodes per core
NTAB = C * NLP        # 50176 table rows (conceptually)
# A/B table split: half A = local tiles [0, 25) (ranks [0, 3200)), half B =
# tiles [25, 49) (ranks [3200, 6272)). Each half is its own Shared tensor,
# allgathered by a single collective as soon as that half is produced; both
# halves stay within int16 gather addressing (< 32768 rows).
TA_T0, TA_T1 = 0, 20         # tiles in half A
TB_T0, TB_T1 = 20, T         # tiles computed in phase-1's B pass
RA = TA_T1 * P               # 2560: half A covers local ranks [0, RA)
RB0 = 17 * P                 # 2176: half B covers local ranks [RB0, NLP)
RB_ = NLP - RB0              # 3968 local ranks in half B
NTA = C * RA                 # 25600 rows in tabA
NTB = C * RB_                # 32768 rows in tabB (max idx 32767 = int16 max)
# ranks [RB0, RA) live in BOTH halves; edges whose src falls in that band
# are flex-assigned to whichever region balances per-(core,tile) blocks.
EL = 128              # padded per-view row length (bf16 -> 256B)
EL2 = 2 * EL          # pair row: [view1 | view2] -> 512B in bf16
KS = 4                # k-slabs for the x @ W1 matmul (500 = 4*125)
KSL = FIN // KS       # 125
SC = 5                # tiles per PSUM superchunk
NSC = (T + SC - 1) // SC  # 10
RBCAP = int(os.environ.get("KERNEL_RBCAP", "49"))

# Phase-1 half-groups (groups of <=4 tiles within each A/B half so each
# half completes contiguously and can stage+allgather early)
HALVES = [(TA_T0, TA_T1), (TB_T0, TB_T1)]

F32 = mybir.dt.float32
BF16 = mybir.dt.bfloat16
F8 = mybir.dt.float8e4
I16 = mybir.dt.int16
TB = BF16
AOp = mybir.AluOpType
SINGLE_PACKET = os.environ.get("KERNEL_SP", "0") == "1"
RB_BUFS = int(os.environ.get("KERNEL_RB_BUFS", "3"))

_cache = {}


def _bc_inner(ap, n):
    """Broadcast AP over a new innermost dim of size n (stride 0)."""
    return bass.AP(ap.tensor, ap.offset, list(ap.ap) + [[0, n]])


def _bc_middle(ap2d, n):
    """[128, F] AP -> [128, n, F] with the middle dim broadcast (stride 0)."""
    a = list(ap2d.ap)
    return bass.AP(ap2d.tensor, ap2d.offset, [a[0], [0, n], a[1]])


def _build(rounds, totb, iw16, hasb1, hasb2, sim_mode=False):
    """rounds: list of (region, b0, [(tile, nb), ...]) shared by both layers.
    totb: total global blocks. Block b's S matrix is s_in[b]."""
    nc = bacc.Bacc(None, num_devices=1 if sim_mode else C,
                   target_bir_lowering=False, debug=False,
                   dynamic_dma_scratch_size=32768)

    # first/last global block per SUPERCHUNK in EXECUTION order (leading
    # region's rounds run first). PSUM start/stop granularity is the whole
    # 2KB bank: exactly one start and one stop per (superchunk, view) bank.
    def mk_first_last(first_region):
        first_b = {}
        last_b = {}
        for pass_reg in (first_region, 1 - first_region):
            for (reg, b0, tl) in rounds:
                if reg != pass_reg:
                    continue
                b = b0
                for (t, nb) in tl:
                    sc = t // SC
                    for i in range(nb):
                        if sc not in first_b:
                            first_b[sc] = b
                        last_b[sc] = b
                        b += 1
        return first_b, last_b

    # ---- inputs ----
    # xt: slab-major transposed x (norm_out folded in on host):
    # xt[v, s, k, t*P + p] = (x_view_v[node(t,p)] * no[node]) [s*KSL + k]
    xt = nc.dram_tensor("xt", [2, KS, KSL, T * P], BF16, kind="ExternalInput")
    idx = nc.dram_tensor("idx", [P, iw16], I16, kind="ExternalInput")
    s_in = nc.dram_tensor("s_in", [P, totb * P], BF16, kind="ExternalInput")
    no_in = nc.dram_tensor("no_in", [P, T], F32, kind="ExternalInput")
    ni_in = nc.dram_tensor("ni_in", [P, T], F32, kind="ExternalInput")
    w1_in = nc.dram_tensor("w1_in", [KS, KSL, H], BF16, kind="ExternalInput")
    w2_in = nc.dram_tensor("w2_in", [H, H], BF16, kind="ExternalInput")
    b1_in = nc.dram_tensor("b1_in", [P, H], F32, kind="ExternalInput")
    a1_in = nc.dram_tensor("a1_in", [P, H], F32, kind="ExternalInput")
    b2_in = nc.dram_tensor("b2_in", [P, H], F32, kind="ExternalInput")
    a2_in = nc.dram_tensor("a2_in", [P, H], F32, kind="ExternalInput")
    wv_in = nc.dram_tensor("wv_in", [P, H + 1], F32, kind="ExternalInput")
    out = nc.dram_tensor("out", [2, NLP], F32, kind="ExternalOutput")
    dbg_mode = os.environ.get("KERNEL_DBG", "0") == "1"
    dbg = (nc.dram_tensor("dbg", [2, T, P, H], F32, kind="ExternalOutput")
           if dbg_mode else None)

    with tile.TileContext(nc) as tc:
        with (
            tc.tile_pool(name="cst", bufs=1) as cst,
            tc.tile_pool(name="wrk", bufs=2) as wrk,
            tc.tile_pool(name="ps", bufs=2, space="PSUM") as ps,
            tc.tile_pool(name="dr", bufs=1, space="DRAM") as dr,
        ):
            # ---- resident constants ----
            idxt = cst.tile([P, iw16], I16)
            nc.sync.dma_start(out=idxt[:], in_=idx[:])
            no_t = cst.tile([P, T], F32)
            nc.sync.dma_start(out=no_t[:], in_=no_in[:])
            ni_t = cst.tile([P, T], F32)
            nc.sync.dma_start(out=ni_t[:], in_=ni_in[:])
            w1t = cst.tile([KSL, KS * H], BF16)
            nc.sync.dma_start(
                out=w1t[:].rearrange("p (s h) -> p s h", s=KS),
                in_=w1_in[:].rearrange("s p h -> p s h"),
            )
            w2t = cst.tile([H, H], BF16)
            nc.sync.dma_start(out=w2t[:], in_=w2_in[:])
            b1r = cst.tile([P, H], F32)
            nc.sync.dma_start(out=b1r[:], in_=b1_in[:])
            a1r = cst.tile([P, H], F32)
            nc.sync.dma_start(out=a1r[:], in_=a1_in[:])
            b2r = cst.tile([P, H], F32)
            nc.sync.dma_start(out=b2r[:], in_=b2_in[:])
            a2r = cst.tile([P, H], F32)
            nc.sync.dma_start(out=a2r[:], in_=a2_in[:])
            wvr = cst.tile([P, H + 1], F32)
            nc.sync.dma_start(out=wvr[:], in_=wv_in[:])
            ident = cst.tile([P, P], F32)
            make_identity(nc, ident[:])
            identb = cst.tile([96, 96], BF16)
            nc.vector.tensor_copy(out=identb[:], in_=ident[0:96, 0:96])

            # Pool touch: advance Pool's vector clock past the idx load
            tch = cst.tile([1, 8], I16, name="tch")
            nc.gpsimd.dma_start(out=tch[0:1, 0:4], in_=idxt[0:1, 0:4])

            # ---- DRAM tables (A/B halves, each one Shared collective) ----
            ypairA = dr.tile([NTA, EL2], TB, addr_space="Shared", name="ypairA")
            ypairB = dr.tile([NTB, EL2], TB, addr_space="Shared", name="ypairB")
            gpairA = dr.tile([NTA, EL2], TB, addr_space="Shared", name="gpairA")
            gpairB = dr.tile([NTB, EL2], TB, addr_space="Shared", name="gpairB")
            psl = dr.tile([NLP, EL2], TB, name="psl")
            gsl = dr.tile([NLP, EL2], TB, name="gsl")

            def allgather_half(sl, tab, half):
                r0, r1 = (0, RA) if half == 0 else (RB0, NLP)
                if sim_mode:
                    nc.sync.dma_start(out=tab[0:r1 - r0, :],
                                      in_=sl[r0:r1, :])
                else:
                    nc.gpsimd.collective_compute(
                        "AllGather", AOp.bypass,
                        replica_groups=[list(range(C))],
                        ins=[sl[r0:r1, :].opt()], outs=[tab[:].opt()],
                    )

            def stage_to_table(stage, sl, t0, nt, v):
                """Write stage tiles [t0, t0+nt) (one view's H cols) into the
                local pair-row slab sl."""
                nc.sync.dma_start(
                    out=bass.AP(sl.tensor,
                                sl.offset + t0 * P * EL2 + v * EL,
                                [[EL2, P], [P * EL2, nt], [1, H]]),
                    in_=stage[:, t0 * H:(t0 + nt) * H].rearrange(
                        "p (t h) -> p t h", h=H),
                )

            # ---- phase 1: both views' tables = (x*no) @ W1 ----
            # W1 slab stationary, x streamed up to 4 tiles (512 node-columns)
            # per matmul -> transposed [96, 512] PSUM; PE-transpose back to
            # node-major per tile. Half A (both views) completes first and
            # allgathers while half B computes.
            ystages = [wrk.tile([P, T * H], TB, tag=f"stage{v}", bufs=1,
                                name=f"ystage_{v}")
                       for v in range(2)]
            for half, (ht0, ht1) in ((0, HALVES[0]), (1, HALVES[1])):
                for v in range(2):
                    ystage = ystages[v]
                    t0 = ht0
                    while t0 < ht1:
                        nt4 = min(ht1, t0 + 4) - t0
                        ncol = nt4 * P
                        yT = ps.tile([96, 4 * P], F32, tag="psv", bufs=6,
                                     space="PSUM", name=f"yT_{half}_{v}_{t0}")
                        for s in range(KS):
                            xsl = wrk.tile([KSL, 4 * P], BF16, tag="xtile",
                                           bufs=4, name=f"xsl_{half}_{v}_{t0}_{s}")
                            nc.sync.dma_start(
                                out=xsl[:, :ncol],
                                in_=xt[v, s][:, t0 * P:t0 * P + ncol])
                            nc.tensor.matmul(
                                out=yT[:, :ncol],
                                lhsT=w1t[:, s * H:(s + 1) * H],
                                rhs=xsl[:, :ncol],
                                start=(s == 0), stop=(s == KS - 1),
                            )
                        ysb = wrk.tile([96, 4 * P], BF16, tag="ysb", bufs=3,
                                       name=f"ysb_{half}_{v}_{t0}")
                        nc.vector.tensor_copy(out=ysb[:, :ncol],
                                              in_=yT[:, :ncol])
                        for i in range(nt4):
                            tp = ps.tile([P, H], BF16, tag="tbank", bufs=1,
                                         space="PSUM", name=f"tp1_{v}_{t0}_{i}")
                            nc.tensor.transpose(
                                out=tp[:], in_=ysb[:, i * P:(i + 1) * P],
                                identity=identb[:])
                            nc.vector.tensor_copy(
                                out=ystage[:, (t0 + i) * H:(t0 + i + 1) * H],
                                in_=tp[:])
                        t0 += nt4
                for vv in range(2):
                    stage_to_table(ystages[vv], psl, ht0, ht1 - ht0, vv)
                if half == 0:
                    allgather_half(psl, ypairA, 0)

            def ag1b():
                allgather_half(psl, ypairB, 1)

            # ---- SpMM over a pair table; S-matrix accumulate on PE ----
            def spmm_pair(tabA, tabB, layer, br, ar, hasb,
                          first_region=0, pending_cc=None):
                first_b, last_b = mk_first_last(first_region)
                gstages = []
                zbs = []
                if layer == 1:
                    for v in range(2):
                        gstages.append(wrk.tile([P, T * H], TB,
                                                tag=f"gstage{v}", bufs=1,
                                                name=f"gst_{v}"))
                else:
                    for v in range(2):
                        zb = wrk.tile([P, T], F32, tag=f"zb{v}", bufs=1,
                                      name=f"zb_{v}")
                        zbs.append(zb)

                def do_prep(reg, b0, tl):
                    """Gather + S-matrix stream for one round."""
                    ntot = sum(nb for (t, nb) in tl)
                    rb = wrk.tile([P, RBCAP * EL2], TB, tag="rb", bufs=RB_BUFS,
                                  name=f"rb_{layer}_{reg}_{b0}")
                    src = tabA[:] if reg == 0 else tabB[:]
                    off16 = b0 * 8
                    nc.gpsimd.dma_gather(
                        out_ap=rb[:, :ntot * EL2].rearrange(
                            "p (b e) -> p b e", e=EL2),
                        in_ap=src,
                        idxs_ap=idxt[:, off16:off16 + ntot * 8],
                        num_idxs=ntot * P, num_idxs_reg=ntot * P,
                        elem_size=EL2,
                        single_packet=SINGLE_PACKET,
                    )
                    sst = wrk.tile([P, RBCAP * P], TB, tag="sst", bufs=RB_BUFS,
                                   name=f"sst_{layer}_{reg}_{b0}")
                    nc.sync.dma_start(
                        out=sst[:, :ntot * P],
                        in_=s_in[:, b0 * P:(b0 + ntot) * P],
                    )
                    return rb, sst

                def do_matmuls(reg, b0, tl, rb, sst, cur_psv):
                    i = 0
                    b = b0
                    for (t, nb) in tl:
                        sc = t // SC
                        for _ in range(nb):
                            for v in range(2):
                                nc.tensor.matmul(
                                    out=cur_psv[v][:, (t - sc * SC) * H:
                                                   (t - sc * SC + 1) * H],
                                    lhsT=sst[:, i * P:(i + 1) * P],
                                    rhs=rb[:, i * EL2 + v * EL:
                                           i * EL2 + v * EL + H],
                                    start=(b == first_b[sc]),
                                    stop=(b == last_b[sc]),
                                )
                            i += 1
                            b += 1

                def do_postop_sc(sc, cur_psv):
                    t0 = sc * SC
                    nt = min(T, t0 + SC) - t0
                    nh = nt * H
                    for v in range(2):
                        hbuf = wrk.tile([P, SC * H], F32, tag="hbuf", bufs=4,
                                        name=f"hbuf_{layer}_{sc}_{v}")
                        nc.vector.tensor_copy(out=hbuf[:, :nh],
                                              in_=cur_psv[v][:, :nh])
                        h3 = hbuf[:, :nh].rearrange("p (t h) -> p t h", h=H)
                        nc.vector.tensor_tensor(
                            out=h3, in0=h3,
                            in1=_bc_inner(ni_t[:, t0:t0 + nt], H), op=AOp.mult)
                        if hasb:
                            nc.vector.tensor_tensor(
                                out=h3, in0=h3, in1=_bc_middle(br[:], nt),
                                op=AOp.add)
                        pos = wrk.tile([P, SC * H], F32, tag="pos", bufs=4,
                                       name=f"pos_{layer}_{sc}_{v}")
                        nc.vector.tensor_scalar(
                            out=pos[:, :nh], in0=hbuf[:, :nh],
                            scalar1=0.0, scalar2=None, op0=AOp.max)
                        nc.vector.tensor_scalar(
                            out=hbuf[:, :nh], in0=hbuf[:, :nh],
                            scalar1=0.0, scalar2=None, op0=AOp.min)
                        nc.vector.tensor_tensor(
                            out=h3, in0=h3, in1=_bc_middle(ar[:], nt),
                            op=AOp.mult)
                        nc.vector.tensor_tensor(
                            out=hbuf[:, :nh], in0=hbuf[:, :nh],
                            in1=pos[:, :nh], op=AOp.add)
                        if layer == 1:
                            if dbg_mode:
                                for i in range(nt):
                                    nc.sync.dma_start(
                                        out=dbg[v, t0 + i],
                                        in_=hbuf[:, i * H:(i + 1) * H])
                            nc.vector.tensor_tensor(
                                out=h3, in0=h3,
                                in1=_bc_inner(no_t[:, t0:t0 + nt], H),
                                op=AOp.mult)
                            gps = ps.tile([P, SC * H], F32, tag="pgp", bufs=1,
                                          space="PSUM")
                            for i in range(nt):
                                tp = ps.tile([H, P], F32, tag="tbank", bufs=1,
                                             space="PSUM")
                                nc.tensor.transpose(
                                    out=tp[:],
                                    in_=hbuf[:, i * H:(i + 1) * H],
                                    identity=ident[:])
                                gsT = wrk.tile([H, P], BF16, tag="gsT", bufs=3)
                                nc.vector.tensor_copy(out=gsT[:], in_=tp[:])
                                nc.tensor.matmul(
                                    out=gps[:, i * H:(i + 1) * H],
                                    lhsT=gsT[:], rhs=w2t[:],
                                    start=(i == 0), stop=(i == nt - 1))
                            nc.vector.tensor_copy(
                                out=gstages[v][:, t0 * H:(t0 + nt) * H],
                                in_=gps[:, :nh])
                        else:
                            nc.vector.tensor_tensor(
                                out=h3, in0=h3, in1=_bc_middle(wvr[:, 0:H], nt),
                                op=AOp.mult)
                            nc.vector.tensor_reduce(
                                out=zbs[v][:, t0:t0 + nt], in_=h3,
                                axis=mybir.AxisListType.X, op=AOp.add,
                            )

                # drive: region-A rounds run up to 2 superchunks AHEAD of the
                # B rounds, so at each layer start the A rounds (whose table
                # half allgathers early) cover the B collective's latency.
                # A superchunk's postops fire once its B rounds finish. In
                # layer 1, each gpair half allgathers as soon as its tiles'
                # postops are done (A after sc 4, B after sc 9).
                fr, orr = first_region, 1 - first_region
                seq = []
                f_next = o_next = 0
                while o_next < NSC:
                    while f_next < NSC and f_next < o_next + 3:
                        seq.append((f_next, fr))
                        f_next += 1
                    seq.append((o_next, orr))
                    o_next += 1
                psvs = {}
                for (sc, r) in seq:
                    if r == orr and pending_cc is not None:
                        pending_cc()
                        pending_cc = None
                    if sc not in psvs:
                        psvs[sc] = [
                            ps.tile([P, 4 * P], F32, tag="psv", bufs=6,
                                    space="PSUM", name=f"psv{v}_{layer}_{sc}")
                            for v in range(2)
                        ]
                    cur_psv = psvs[sc]
                    for (reg, b0, tl) in rounds:
                        if reg != r or tl[0][0] // SC != sc:
                            continue
                        rb, sst = do_prep(reg, b0, tl)
                        do_matmuls(reg, b0, tl, rb, sst, cur_psv)
                    if r == orr:
                        do_postop_sc(sc, cur_psv)
                        del psvs[sc]
                        if layer == 1 and sc == 3:
                            for v in range(2):
                                stage_to_table(gstages[v], gsl, TA_T0,
                                               TA_T1 - TA_T0, v)
                            allgather_half(gsl, gpairA, 0)
                        if layer == 1 and sc >= 4:
                            t0s = sc * SC
                            nts = min(T, t0s + SC) - t0s
                            for v in range(2):
                                stage_to_table(gstages[v], gsl, t0s, nts, v)

                if layer != 1:
                    for v in range(2):
                        nc.vector.tensor_scalar(
                            out=zbs[v][:], in0=zbs[v][:],
                            scalar1=wvr[:, H:H + 1], scalar2=None,
                            op0=AOp.add,
                        )
                        nc.sync.dma_start(
                            out=bass.AP(out, v * NLP, [[1, P], [P, T]]),
                            in_=zbs[v][:],
                        )
                return gstages

            gst1 = spmm_pair(ypairA, ypairB, 1, b1r, a1r, hasb1,
                             first_region=0, pending_cc=ag1b)

            def ag2b():
                allgather_half(gsl, gpairB, 1)

            spmm_pair(gpairA, gpairB, 2, b2r, a2r, hasb2,
                      first_region=0, pending_cc=ag2b)

    nc.finalize()
    return nc


def _wrap16(vals):
    """[n] int array -> wrapped [128, n/16] int16 (replicated across 8 Q7 cores)."""
    n = vals.shape[-1]
    assert n % 16 == 0
    w = vals.reshape(n // 16, 16).T.astype(np.int16)
    return np.tile(w, (8, 1))


def _host_plan(src, dst):
    """Builds l-space mapping, per-edge region, dense blocks, rounds."""
    deg_in = np.bincount(dst, minlength=N)

    # snake-balance nodes over the 392 (t-major, c-minor) groups so every
    # (core, tile) has ~equal total in-degree -> equal block counts
    order = np.argsort(-deg_in, kind="stable")
    ranked = np.full((C, T * P), -1, np.int64)
    fill = np.zeros((T, C), np.int64)
    gsizes = np.full((T, C), P, np.int64)
    gsizes[T - 1, :] = NL - (T - 1) * P  # 106 real slots in last tile
    pos = 0
    rnd = 0
    while pos < N:
        active = [(t, c) for t in range(T) for c in range(C)
                  if fill[t, c] < gsizes[t, c]]
        if rnd % 2 == 1:
            active = active[::-1]
        for (t, c) in active:
            if pos >= N:
                break
            node = order[pos]
            ranked[c, t * P + fill[t, c]] = node
            fill[t, c] += 1
            pos += 1
        rnd += 1
    Lmap = np.empty(N, np.int64)   # node -> table row (c-major, rank-minor)
    Cmap = np.empty(N, np.int64)   # node -> owning core
    Rmap = np.empty(N, np.int64)   # node -> local rank
    for c in range(C):
        m = ranked[c] >= 0
        rk_ = np.where(m)[0]
        nodes = ranked[c][m]
        Lmap[nodes] = c * NLP + rk_
        Cmap[nodes] = c
        Rmap[nodes] = rk_

    c_e = Cmap[dst]
    r_e = Rmap[dst]
    t_e = r_e // P
    p_e = r_e % P

    # per-edge region: forced outside the overlap band [RB0, RA); edges whose
    # src rank falls inside it are flex-assigned to balance per-(c,t) counts
    src_c = Cmap[src]
    src_r = Rmap[src]
    forcedA = src_r < RB0
    forcedB = src_r >= RA
    flex = ~(forcedA | forcedB)
    gid = c_e * T + t_e
    nA = np.bincount(gid[forcedA], minlength=C * T)
    nF = np.bincount(gid[flex], minlength=C * T)
    ntot = np.bincount(gid, minlength=C * T)
    addA = np.clip((ntot + 1) // 2 - nA, 0, nF)
    fidx = np.where(flex)[0]
    fg = gid[fidx]
    forder = np.argsort(fg, kind="stable")
    fo = fidx[forder]
    fgo = fg[forder]
    gstarts = np.concatenate(([0], np.cumsum(np.bincount(fgo, minlength=C * T))))[:-1]
    rank_in_g = np.arange(len(fo)) - gstarts[fgo]
    region = np.ones(E, np.int8)
    region[forcedA] = 0
    region[fo[rank_in_g < addA[fgo]]] = 0
    # row index within the half's table: A: c*RA + rank; B: c*RB_ + rank-RB0
    lsrc_half = np.where(region == 0, src_c * RA + src_r,
                         src_c * RB_ + (src_r - RB0))

    # per (c, t, r) counts and shared block counts
    cnt = np.zeros((C, T, 2), np.int64)
    np.add.at(cnt, (c_e, t_e, region), 1)
    nblk = np.maximum(1, (cnt.max(axis=0) + P - 1) // P)  # [T, 2]

    # global block order: superchunk-major, region inner, tile inner
    rounds = []
    binfo = []  # per global block: (tile, region)
    b0_of = {}  # (t, r) -> first global block idx
    gb = 0
    for sc in range(NSC):
        tiles = range(sc * SC, min(T, (sc + 1) * SC))
        for r in range(2):
            run = []
            for t in tiles:
                b0_of[(t, r)] = gb
                run.append((t, int(nblk[t, r])))
                for _ in range(int(nblk[t, r])):
                    binfo.append((t, r))
                gb += int(nblk[t, r])
            # split run into rounds of <= RBCAP blocks
            cur = []
            cb = 0
            curb0 = b0_of[(run[0][0], r)]
            for (t, nb) in run:
                left = nb
                while left > 0:
                    take = min(left, RBCAP - cb)
                    if take == 0:
                        rounds.append((r, curb0, cur))
                        curb0 += cb
                        cur = []
                        cb = 0
                        continue
                    cur.append((t, take))
                    cb += take
                    left -= take
            if cur:
                rounds.append((r, curb0, cur))
    totb = gb

    # place each edge: (global block, row)
    eorder = np.lexsort((r_e, region.astype(np.int64), t_e, c_e))
    # recompute placement per (c,t,r) run using cumulative position
    ce_s = c_e[eorder]
    te_s = t_e[eorder]
    re_s = region[eorder].astype(np.int64)
    key = (ce_s * T + te_s) * 2 + re_s
    kcnt = np.bincount(key, minlength=C * T * 2)
    kstart = np.concatenate(([0], np.cumsum(kcnt)))[:-1]
    pos_in_run = np.arange(E) - kstart[key]
    b0_arr = np.zeros((T, 2), np.int64)
    for t in range(T):
        for r in range(2):
            b0_arr[t, r] = b0_of[(t, r)]
    blk = b0_arr[te_s, re_s] + pos_in_run // P
    row = pos_in_run % P
    loc = lsrc_half[eorder].astype(np.int16)
    pe_s = p_e[eorder]

    # idx array: [totb*128] int16, pad = 0
    iw16 = totb * 8
    idxflat = np.zeros(totb * P, np.int16)
    idxflat[blk * P + row] = loc
    IDX = np.empty((C, P, iw16), np.int16)
    # per-core idx: only this core's edges (others' blocks stay pad)
    for c in range(C):
        m = ce_s == c
        f = np.zeros(totb * P, np.int16)
        f[blk[m] * P + row[m]] = loc[m]
        IDX[c] = _wrap16(f)

    # S matrices, device layout [C, P(row), totb*P]: sst[r, b*P+c] = S[b,r,c]
    S = np.zeros((C, P, totb * P), ml_dtypes.bfloat16)
    one = ml_dtypes.bfloat16(1.0)
    for c in range(C):
        m = ce_s == c
        S[c, row[m], blk[m] * P + pe_s[m]] = one

    return ranked, Lmap, rounds, totb, iw16, IDX, S


def kernel(x, src, dst, perm, W1, b1, a1, W2, b2, a2, Wm, bm):
    x = np.ascontiguousarray(np.asarray(x, np.float32))
    src = np.asarray(src, np.int64)
    dst = np.asarray(dst, np.int64)
    perm = np.asarray(perm, np.int64)
    W1 = np.asarray(W1, np.float32); W2 = np.asarray(W2, np.float32)
    Wm = np.asarray(Wm, np.float32)
    b1 = np.asarray(b1, np.float32); b2 = np.asarray(b2, np.float32)
    a1 = np.asarray(a1, np.float32); a2 = np.asarray(a2, np.float32)
    bm = np.asarray(bm, np.float32)

    deg_out = np.bincount(src, minlength=N)
    deg_in = np.bincount(dst, minlength=N)
    norm_out = np.maximum(deg_out, 1).astype(np.float32) ** -0.5
    norm_in = np.maximum(deg_in, 1).astype(np.float32) ** -0.5

    ranked, Lmap, rounds, totb, iw16, IDX, S = _host_plan(src, dst)

    def tile_pt(vals_pad):
        return np.ascontiguousarray(vals_pad.reshape(C, T, P).transpose(0, 2, 1))

    rmask = ranked >= 0
    rk = np.where(rmask, ranked, 0)
    no_p = np.where(rmask, norm_out[rk], 0).astype(np.float32)
    ni_p = np.where(rmask, norm_in[rk], 0).astype(np.float32)
    NO = tile_pt(no_p); NI = tile_pt(ni_p)

    # norm_out folded into x on the host (both views scale by the table
    # OWNER row's norm_out)
    xp = np.zeros((2, C, NLP, FIN), ml_dtypes.bfloat16)
    for c in range(C):
        m = rmask[c]
        nodes = rk[c][m]
        sc_ = norm_out[nodes][:, None]
        xp[0, c, m] = (x[nodes] * sc_).astype(ml_dtypes.bfloat16)
        xp[1, c, m] = (x[perm[nodes]] * sc_).astype(ml_dtypes.bfloat16)
    # device layout: xt[v, s, k, t*P + p] = x_view_v[node(t,p), s*KSL + k]
    XT = np.ascontiguousarray(
        xp.reshape(2, C, T, P, KS, KSL).transpose(1, 0, 4, 5, 2, 3)
        .reshape(C, 2, KS, KSL, T * P))

    w1s = np.ascontiguousarray(W1.reshape(KS, KSL, H)).astype(ml_dtypes.bfloat16)
    rep = lambda v: np.ascontiguousarray(np.tile(v[None, :], (P, 1)))
    wv = Wm.sum(axis=1)
    wvb = np.concatenate([wv, [bm.sum()]]).astype(np.float32)
    hasb1 = bool(np.any(b1)); hasb2 = bool(np.any(b2))

    rkey = tuple((r, b0, tuple(tl)) for (r, b0, tl) in rounds)
    ck = (hash(rkey), totb, iw16, hasb1, hasb2,
          os.environ.get("KERNEL_DBG", "0"))
    if ck not in _cache:
        _cache[ck] = _build(rounds, totb, iw16, hasb1, hasb2)
    nc = _cache[ck]

    shared = {
        "w1_in": w1s, "w2_in": W2.astype(ml_dtypes.bfloat16),
        "b1_in": rep(b1), "a1_in": rep(a1),
        "b2_in": rep(b2), "a2_in": rep(a2), "wv_in": rep(wvb),
    }
    in_maps = []
    for c in range(C):
        m = dict(shared)
        m.update({
            "xt": XT[c], "idx": IDX[c], "s_in": S[c],
            "no_in": NO[c], "ni_in": NI[c],
        })
        in_maps.append(m)

    trace = os.environ.get("KERNEL_TRACE", "0") == "1"
    res = run_bass_kernel_spmd(nc, in_maps, core_ids=list(range(C)), trace=trace)
    if res.exec_time_ns is not None:
        print(f"HW exec time: {res.exec_time_ns} ns")
        kernel.last_exec_time_ns = res.exec_time_ns

    z = np.empty((2, N), np.float32)
    for c in range(C):
        o = res.results[c]["out"]
        m = rmask[c]
        z[0, ranked[c][m]] = o[0][m]
        z[1, ranked[c][m]] = o[1][m]
    return np.concatenate([z[0], z[1]]).astype(np.float32)



# revision 52
# speedup vs baseline: 1.0490x; 1.0376x over previous
"""DinkNet GNN (2-layer GraphConv encoder, two views) on 8 Trainium2 NeuronCores.

Strategy (self-contained; shapes hardcoded for the nn_DinkNet_dgl problem):
  - Nodes sharded across 8 cores; l(g) = core*6272 + rank. Graph preprocessing
    (degree snake-balance, edge blocks, selection matrices) happens on the
    host; all heavy FP work runs on the NeuronCores.
  - Math folding: (x*no) @ W1 = no * (x@W1); the corrupted view's table is
    computed directly from host-gathered x[perm] rows (no device-side perm
    gather); final (z@Wm+bm).sum(1) = h2 @ Wm.sum(1) + bm.sum().
  - SpMM over 800k edges: per-core edges packed DENSELY into 128-row blocks
    (tile-major); one GPSIMD dma_gather per (5-tile superchunk, region)
    fetches the src pair-rows (512B each, both views interleaved); a per-block
    one-hot selection matrix S (streamed bf16 from HBM via HWDGE) routes and
    segment-sums rows into the dst tile's PSUM region on the TensorEngine.
    Dense packing avoids the per-(dst,slot) padding of slot-structured
    gathers; desc-gen on the Q7 cores is the bottleneck, so fewer gather
    descriptors = faster.
  - The table is split into two Shared tensors: A = all cores' local ranks
    [0, 3200) and B = ranks [2176, 6272) (overlapping flex band so edges can
    be region-balanced). Each half allgathers with one collective as soon as
    it is produced (A at mid-phase-1 / after L1 superchunk 4), so collectives
    overlap compute/desc-gen; each layer's round sequence runs region-A
    rounds up to 3 superchunks ahead so they cover the B collective's
    latency. Both halves stay within int16 gather addressing (<= 32768 rows).
  - Phase 1 streams x through W1-stationary 512-column matmuls (transposed
    [96, cols] PSUM) and PE-transposes per tile back to node-major; norm_out
    is folded into x on the host.
"""
import os
import numpy as np
import ml_dtypes

import concourse.bass as bass
import concourse.bacc as bacc
import concourse.mybir as mybir
import concourse.tile as tile
from concourse.bass_utils import run_bass_kernel_spmd
from concourse.masks import make_identity

# Problem shapes (hardcoded per contract).
N, E, FIN, H = 50000, 800000, 500, 96
C = 8                 # cores
P = 128               # partitions
NL = N // C           # 6250 real nodes per core
T = (NL + P - 1) // P # 49 rank tiles per core
NLP = T * P           # 6272 padded nodes per core
NTAB = C * NLP        # 50176 table rows (conceptually)
# A/B table split: half A = local tiles [0, 25) (ranks [0, 3200)), half B =
# tiles [25, 49) (ranks [3200, 6272)). Each half is its own Shared tensor,
# allgathered by a single collective as soon as that half is produced; both
# halves stay within int16 gather addressing (< 32768 rows).
TA_T0, TA_T1 = 0, 25         # tiles in half A
TB_T0, TB_T1 = 25, T         # tiles computed in phase-1's B pass
RA = TA_T1 * P               # 3200: half A covers local ranks [0, RA)
RB0 = 17 * P                 # 2176: half B covers local ranks [RB0, NLP)
RB_ = NLP - RB0              # 3968 local ranks in half B
NTA = C * RA                 # 25600 rows in tabA
NTB = C * RB_                # 32768 rows in tabB (max idx 32767 = int16 max)
# ranks [RB0, RA) live in BOTH halves; edges whose src falls in that band
# are flex-assigned to whichever region balances per-(core,tile) blocks.
EL = 128              # padded per-view row length (bf16 -> 256B)
EL2 = 2 * EL          # pair row: [view1 | view2] -> 512B in bf16
KS = 4                # k-slabs for the x @ W1 matmul (500 = 4*125)
KSL = FIN // KS       # 125
SC = 5                # tiles per PSUM superchunk
NSC = (T + SC - 1) // SC  # 10
RBCAP = int(os.environ.get("KERNEL_RBCAP", "49"))

# Phase-1 half-groups (groups of <=4 tiles within each A/B half so each
# half completes contiguously and can stage+allgather early)
HALVES = [(TA_T0, TA_T1), (TB_T0, TB_T1)]

F32 = mybir.dt.float32
BF16 = mybir.dt.bfloat16
F8 = mybir.dt.float8e4
I16 = mybir.dt.int16
TB = BF16
AOp = mybir.AluOpType
SINGLE_PACKET = os.environ.get("KERNEL_SP", "0") == "1"
RB_BUFS = int(os.environ.get("KERNEL_RB_BUFS", "3"))

_cache = {}


def _bc_inner(ap, n):
    """Broadcast AP over a new innermost dim of size n (stride 0)."""
    return bass.AP(ap.tensor, ap.offset, list(ap.ap) + [[0, n]])


def _bc_middle(ap2d, n):
    """[128, F] AP -> [128, n, F] with the middle dim broadcast (stride 0)."""
    a = list(ap2d.ap)
    return bass.AP(ap2d.tensor, ap2d.offset, [a[0], [0, n], a[1]])


def _build(rounds, totb, iw16, hasb1, hasb2, sim_mode=False):
    """rounds: list of (region, b0, [(tile, nb), ...]) shared by both layers.
    totb: total global blocks. Block b's S matrix is s_in[b]."""
    nc = bacc.Bacc(None, num_devices=1 if sim_mode else C,
                   target_bir_lowering=False, debug=False,
                   dynamic_dma_scratch_size=32768)

    # first/last global block per SUPERCHUNK in EXECUTION order (leading
    # region's rounds run first). PSUM start/stop granularity is the whole
    # 2KB bank: exactly one start and one stop per (superchunk, view) bank.
    def mk_first_last(first_region):
        first_b = {}
        last_b = {}
        for pass_reg in (first_region, 1 - first_region):
            for (reg, b0, tl) in rounds:
                if reg != pass_reg:
                    continue
                b = b0
                for (t, nb) in tl:
                    sc = t // SC
                    for i in range(nb):
                        if sc not in first_b:
                            first_b[sc] = b
                        last_b[sc] = b
                        b += 1
        return first_b, last_b

    # ---- inputs ----
    # xt: slab-major transposed x (norm_out folded in on host):
    # xt[v, s, k, t*P + p] = (x_view_v[node(t,p)] * no[node]) [s*KSL + k]
    xt = nc.dram_tensor("xt", [2, KS, KSL, T * P], BF16, kind="ExternalInput")
    idx = nc.dram_tensor("idx", [P, iw16], I16, kind="ExternalInput")
    s_in = nc.dram_tensor("s_in", [P, totb * P], BF16, kind="ExternalInput")
    no_in = nc.dram_tensor("no_in", [P, T], F32, kind="ExternalInput")
    ni_in = nc.dram_tensor("ni_in", [P, T], F32, kind="ExternalInput")
    w1_in = nc.dram_tensor("w1_in", [KS, KSL, H], BF16, kind="ExternalInput")
    w2_in = nc.dram_tensor("w2_in", [H, H], BF16, kind="ExternalInput")
    b1_in = nc.dram_tensor("b1_in", [P, H], F32, kind="ExternalInput")
    a1_in = nc.dram_tensor("a1_in", [P, H], F32, kind="ExternalInput")
    b2_in = nc.dram_tensor("b2_in", [P, H], F32, kind="ExternalInput")
    a2_in = nc.dram_tensor("a2_in", [P, H], F32, kind="ExternalInput")
    wv_in = nc.dram_tensor("wv_in", [P, H + 1], F32, kind="ExternalInput")
    out = nc.dram_tensor("out", [2, NLP], F32, kind="ExternalOutput")
    dbg_mode = os.environ.get("KERNEL_DBG", "0") == "1"
    dbg = (nc.dram_tensor("dbg", [2, T, P, H], F32, kind="ExternalOutput")
           if dbg_mode else None)

    with tile.TileContext(nc) as tc:
        with (
            tc.tile_pool(name="cst", bufs=1) as cst,
            tc.tile_pool(name="wrk", bufs=2) as wrk,
            tc.tile_pool(name="ps", bufs=2, space="PSUM") as ps,
            tc.tile_pool(name="dr", bufs=1, space="DRAM") as dr,
        ):
            # ---- resident constants ----
            idxt = cst.tile([P, iw16], I16)
            nc.sync.dma_start(out=idxt[:], in_=idx[:])
            no_t = cst.tile([P, T], F32)
            nc.sync.dma_start(out=no_t[:], in_=no_in[:])
            ni_t = cst.tile([P, T], F32)
            nc.sync.dma_start(out=ni_t[:], in_=ni_in[:])
            w1t = cst.tile([KSL, KS * H], BF16)
            nc.sync.dma_start(
                out=w1t[:].rearrange("p (s h) -> p s h", s=KS),
                in_=w1_in[:].rearrange("s p h -> p s h"),
            )
            w2t = cst.tile([H, H], BF16)
            nc.sync.dma_start(out=w2t[:], in_=w2_in[:])
            b1r = cst.tile([P, H], F32)
            nc.sync.dma_start(out=b1r[:], in_=b1_in[:])
            a1r = cst.tile([P, H], F32)
            nc.sync.dma_start(out=a1r[:], in_=a1_in[:])
            b2r = cst.tile([P, H], F32)
            nc.sync.dma_start(out=b2r[:], in_=b2_in[:])
            a2r = cst.tile([P, H], F32)
            nc.sync.dma_start(out=a2r[:], in_=a2_in[:])
            wvr = cst.tile([P, H + 1], F32)
            nc.sync.dma_start(out=wvr[:], in_=wv_in[:])
            ident = cst.tile([P, P], F32)
            make_identity(nc, ident[:])
            identb = cst.tile([96, 96], BF16)
            nc.vector.tensor_copy(out=identb[:], in_=ident[0:96, 0:96])

            # Pool touch: advance Pool's vector clock past the idx load
            tch = cst.tile([1, 8], I16, name="tch")
            nc.gpsimd.dma_start(out=tch[0:1, 0:4], in_=idxt[0:1, 0:4])

            # ---- DRAM tables (A/B halves, each one Shared collective) ----
            ypairA = dr.tile([NTA, EL2], TB, addr_space="Shared", name="ypairA")
            ypairB = dr.tile([NTB, EL2], TB, addr_space="Shared", name="ypairB")
            gpairA = dr.tile([NTA, EL2], TB, addr_space="Shared", name="gpairA")
            gpairB = dr.tile([NTB, EL2], TB, addr_space="Shared", name="gpairB")
            psl = dr.tile([NLP, EL2], TB, name="psl")
            gsl = dr.tile([NLP, EL2], TB, name="gsl")

            def allgather_half(sl, tab, half):
                r0, r1 = (0, RA) if half == 0 else (RB0, NLP)
                if sim_mode:
                    nc.sync.dma_start(out=tab[0:r1 - r0, :],
                                      in_=sl[r0:r1, :])
                else:
                    nc.gpsimd.collective_compute(
                        "AllGather", AOp.bypass,
                        replica_groups=[list(range(C))],
                        ins=[sl[r0:r1, :].opt()], outs=[tab[:].opt()],
                    )

            def stage_to_table(stage, sl, t0, nt, v):
                """Write stage tiles [t0, t0+nt) (one view's H cols) into the
                local pair-row slab sl."""
                nc.sync.dma_start(
                    out=bass.AP(sl.tensor,
                                sl.offset + t0 * P * EL2 + v * EL,
                                [[EL2, P], [P * EL2, nt], [1, H]]),
                    in_=stage[:, t0 * H:(t0 + nt) * H].rearrange(
                        "p (t h) -> p t h", h=H),
                )

            # ---- phase 1: both views' tables = (x*no) @ W1 ----
            # W1 slab stationary, x streamed up to 4 tiles (512 node-columns)
            # per matmul -> transposed [96, 512] PSUM; PE-transpose back to
            # node-major per tile. Half A (both views) completes first and
            # allgathers while half B computes.
            ystages = [wrk.tile([P, T * H], TB, tag=f"stage{v}", bufs=1,
                                name=f"ystage_{v}")
                       for v in range(2)]
            for half, (ht0, ht1) in ((0, HALVES[0]), (1, HALVES[1])):
                for v in range(2):
                    ystage = ystages[v]
                    t0 = ht0
                    while t0 < ht1:
                        nt4 = min(ht1, t0 + 4) - t0
                        ncol = nt4 * P
                        yT = ps.tile([96, 4 * P], F32, tag="psv", bufs=6,
                                     space="PSUM", name=f"yT_{half}_{v}_{t0}")
                        for s in range(KS):
                            xsl = wrk.tile([KSL, 4 * P], BF16, tag="xtile",
                                           bufs=4, name=f"xsl_{half}_{v}_{t0}_{s}")
                            nc.sync.dma_start(
                                out=xsl[:, :ncol],
                                in_=xt[v, s][:, t0 * P:t0 * P + ncol])
                            nc.tensor.matmul(
                                out=yT[:, :ncol],
                                lhsT=w1t[:, s * H:(s + 1) * H],
                                rhs=xsl[:, :ncol],
                                start=(s == 0), stop=(s == KS - 1),
                            )
                        ysb = wrk.tile([96, 4 * P], BF16, tag="ysb", bufs=3,
                                       name=f"ysb_{half}_{v}_{t0}")
                        nc.vector.tensor_copy(out=ysb[:, :ncol],
                                              in_=yT[:, :ncol])
                        for i in range(nt4):
                            tp = ps.tile([P, H], BF16, tag="tbank", bufs=1,
                                         space="PSUM", name=f"tp1_{v}_{t0}_{i}")
                            nc.tensor.transpose(
                                out=tp[:], in_=ysb[:, i * P:(i + 1) * P],
                                identity=identb[:])
                            nc.vector.tensor_copy(
                                out=ystage[:, (t0 + i) * H:(t0 + i + 1) * H],
                                in_=tp[:])
                        t0 += nt4
                for vv in range(2):
                    stage_to_table(ystages[vv], psl, ht0, ht1 - ht0, vv)
                if half == 0:
                    allgather_half(psl, ypairA, 0)

            def ag1b():
                allgather_half(psl, ypairB, 1)

            # ---- SpMM over a pair table; S-matrix accumulate on PE ----
            def spmm_pair(tabA, tabB, layer, br, ar, hasb,
                          first_region=0, pending_cc=None):
                first_b, last_b = mk_first_last(first_region)
                gstages = []
                zbs = []
                if layer == 1:
                    for v in range(2):
                        gstages.append(wrk.tile([P, T * H], TB,
                                                tag=f"gstage{v}", bufs=1,
                                                name=f"gst_{v}"))
                else:
                    for v in range(2):
                        zb = wrk.tile([P, T], F32, tag=f"zb{v}", bufs=1,
                                      name=f"zb_{v}")
                        zbs.append(zb)

                def do_prep(reg, b0, tl):
                    """Gather + S-matrix stream for one round."""
                    ntot = sum(nb for (t, nb) in tl)
                    rb = wrk.tile([P, RBCAP * EL2], TB, tag="rb", bufs=RB_BUFS,
                                  name=f"rb_{layer}_{reg}_{b0}")
                    src = tabA[:] if reg == 0 else tabB[:]
                    off16 = b0 * 8
                    nc.gpsimd.dma_gather(
                        out_ap=rb[:, :ntot * EL2].rearrange(
                            "p (b e) -> p b e", e=EL2),
                        in_ap=src,
                        idxs_ap=idxt[:, off16:off16 + ntot * 8],
                        num_idxs=ntot * P, num_idxs_reg=ntot * P,
                        elem_size=EL2,
                        single_packet=SINGLE_PACKET,
                    )
                    sst = wrk.tile([P, RBCAP * P], TB, tag="sst", bufs=RB_BUFS,
                                   name=f"sst_{layer}_{reg}_{b0}")
                    nc.sync.dma_start(
                        out=sst[:, :ntot * P],
                        in_=s_in[:, b0 * P:(b0 + ntot) * P],
                    )
                    return rb, sst

                def do_matmuls(reg, b0, tl, rb, sst, cur_psv):
                    i = 0
                    b = b0
                    for (t, nb) in tl:
                        sc = t // SC
                        for _ in range(nb):
                            for v in range(2):
                                nc.tensor.matmul(
                                    out=cur_psv[v][:, (t - sc * SC) * H:
                                                   (t - sc * SC + 1) * H],
                                    lhsT=sst[:, i * P:(i + 1) * P],
                                    rhs=rb[:, i * EL2 + v * EL:
                                           i * EL2 + v * EL + H],
                                    start=(b == first_b[sc]),
                                    stop=(b == last_b[sc]),
                                )
                            i += 1
                            b += 1

                def do_postop_sc(sc, cur_psv):
                    t0 = sc * SC
                    nt = min(T, t0 + SC) - t0
                    nh = nt * H
                    for v in range(2):
                        hbuf = wrk.tile([P, SC * H], F32, tag="hbuf", bufs=4,
                                        name=f"hbuf_{layer}_{sc}_{v}")
                        nc.vector.tensor_copy(out=hbuf[:, :nh],
                                              in_=cur_psv[v][:, :nh])
                        h3 = hbuf[:, :nh].rearrange("p (t h) -> p t h", h=H)
                        nc.vector.tensor_tensor(
                            out=h3, in0=h3,
                            in1=_bc_inner(ni_t[:, t0:t0 + nt], H), op=AOp.mult)
                        if hasb:
                            nc.vector.tensor_tensor(
                                out=h3, in0=h3, in1=_bc_middle(br[:], nt),
                                op=AOp.add)
                        pos = wrk.tile([P, SC * H], F32, tag="pos", bufs=4,
                                       name=f"pos_{layer}_{sc}_{v}")
                        nc.vector.tensor_scalar(
                            out=pos[:, :nh], in0=hbuf[:, :nh],
                            scalar1=0.0, scalar2=None, op0=AOp.max)
                        nc.vector.tensor_scalar(
                            out=hbuf[:, :nh], in0=hbuf[:, :nh],
                            scalar1=0.0, scalar2=None, op0=AOp.min)
                        nc.vector.tensor_tensor(
                            out=h3, in0=h3, in1=_bc_middle(ar[:], nt),
                            op=AOp.mult)
                        nc.vector.tensor_tensor(
                            out=hbuf[:, :nh], in0=hbuf[:, :nh],
                            in1=pos[:, :nh], op=AOp.add)
                        if layer == 1:
                            if dbg_mode:
                                for i in range(nt):
                                    nc.sync.dma_start(
                                        out=dbg[v, t0 + i],
                                        in_=hbuf[:, i * H:(i + 1) * H])
                            nc.vector.tensor_tensor(
                                out=h3, in0=h3,
                                in1=_bc_inner(no_t[:, t0:t0 + nt], H),
                                op=AOp.mult)
                            gps = ps.tile([P, SC * H], F32, tag="pgp", bufs=1,
                                          space="PSUM")
                            for i in range(nt):
                                tp = ps.tile([H, P], F32, tag="tbank", bufs=1,
                                             space="PSUM")
                                nc.tensor.transpose(
                                    out=tp[:],
                                    in_=hbuf[:, i * H:(i + 1) * H],
                                    identity=ident[:])
                                gsT = wrk.tile([H, P], BF16, tag="gsT", bufs=3)
                                nc.vector.tensor_copy(out=gsT[:], in_=tp[:])
                                nc.tensor.matmul(
                                    out=gps[:, i * H:(i + 1) * H],
                                    lhsT=gsT[:], rhs=w2t[:],
                                    start=(i == 0), stop=(i == nt - 1))
                            nc.vector.tensor_copy(
                                out=gstages[v][:, t0 * H:(t0 + nt) * H],
                                in_=gps[:, :nh])
                        else:
                            nc.vector.tensor_tensor(
                                out=h3, in0=h3, in1=_bc_middle(wvr[:, 0:H], nt),
                                op=AOp.mult)
                            nc.vector.tensor_reduce(
                                out=zbs[v][:, t0:t0 + nt], in_=h3,
                                axis=mybir.AxisListType.X, op=AOp.add,
                            )

                # drive: region-A rounds run up to 2 superchunks AHEAD of the
                # B rounds, so at each layer start the A rounds (whose table
                # half allgathers early) cover the B collective's latency.
                # A superchunk's postops fire once its B rounds finish. In
                # layer 1, each gpair half allgathers as soon as its tiles'
                # postops are done (A after sc 4, B after sc 9).
                fr, orr = first_region, 1 - first_region
                seq = []
                f_next = o_next = 0
                while o_next < NSC:
                    while f_next < NSC and f_next < o_next + 3:
                        seq.append((f_next, fr))
                        f_next += 1
                    seq.append((o_next, orr))
                    o_next += 1
                psvs = {}
                for (sc, r) in seq:
                    if r == orr and pending_cc is not None:
                        pending_cc()
                        pending_cc = None
                    if sc not in psvs:
                        psvs[sc] = [
                            ps.tile([P, 4 * P], F32, tag="psv", bufs=6,
                                    space="PSUM", name=f"psv{v}_{layer}_{sc}")
                            for v in range(2)
                        ]
                    cur_psv = psvs[sc]
                    for (reg, b0, tl) in rounds:
                        if reg != r or tl[0][0] // SC != sc:
                            continue
                        rb, sst = do_prep(reg, b0, tl)
                        do_matmuls(reg, b0, tl, rb, sst, cur_psv)
                    if r == orr:
                        do_postop_sc(sc, cur_psv)
                        del psvs[sc]
                        if layer == 1 and sc == 4:
                            for v in range(2):
                                stage_to_table(gstages[v], gsl, TA_T0,
                                               TA_T1 - TA_T0, v)
                            allgather_half(gsl, gpairA, 0)
                        if layer == 1 and sc >= 5:
                            t0s = sc * SC
                            nts = min(T, t0s + SC) - t0s
                            for v in range(2):
                                stage_to_table(gstages[v], gsl, t0s, nts, v)

                if layer != 1:
                    for v in range(2):
                        nc.vector.tensor_scalar(
                            out=zbs[v][:], in0=zbs[v][:],
                            scalar1=wvr[:, H:H + 1], scalar2=None,
                            op0=AOp.add,
                        )
                        nc.sync.dma_start(
                            out=bass.AP(out, v * NLP, [[1, P], [P, T]]),
                            in_=zbs[v][:],
                        )
                return gstages

            gst1 = spmm_pair(ypairA, ypairB, 1, b1r, a1r, hasb1,
                             first_region=0, pending_cc=ag1b)

            def ag2b():
                allgather_half(gsl, gpairB, 1)

            spmm_pair(gpairA, gpairB, 2, b2r, a2r, hasb2,
                      first_region=0, pending_cc=ag2b)

    nc.finalize()
    return nc


def _wrap16(vals):
    """[n] int array -> wrapped [128, n/16] int16 (replicated across 8 Q7 cores)."""
    n = vals.shape[-1]
    assert n % 16 == 0
    w = vals.reshape(n // 16, 16).T.astype(np.int16)
    return np.tile(w, (8, 1))


def _host_plan(src, dst):
    """Builds l-space mapping, per-edge region, dense blocks, rounds."""
    deg_in = np.bincount(dst, minlength=N)

    # snake-balance nodes over the 392 (t-major, c-minor) groups so every
    # (core, tile) has ~equal total in-degree -> equal block counts
    order = np.argsort(-deg_in, kind="stable")
    ranked = np.full((C, T * P), -1, np.int64)
    fill = np.zeros((T, C), np.int64)
    gsizes = np.full((T, C), P, np.int64)
    gsizes[T - 1, :] = NL - (T - 1) * P  # 106 real slots in last tile
    pos = 0
    rnd = 0
    while pos < N:
        active = [(t, c) for t in range(T) for c in range(C)
                  if fill[t, c] < gsizes[t, c]]
        if rnd % 2 == 1:
            active = active[::-1]
        for (t, c) in active:
            if pos >= N:
                break
            node = order[pos]
            ranked[c, t * P + fill[t, c]] = node
            fill[t, c] += 1
            pos += 1
        rnd += 1
    Lmap = np.empty(N, np.int64)   # node -> table row (c-major, rank-minor)
    Cmap = np.empty(N, np.int64)   # node -> owning core
    Rmap = np.empty(N, np.int64)   # node -> local rank
    for c in range(C):
        m = ranked[c] >= 0
        rk_ = np.where(m)[0]
        nodes = ranked[c][m]
        Lmap[nodes] = c * NLP + rk_
        Cmap[nodes] = c
        Rmap[nodes] = rk_

    c_e = Cmap[dst]
    r_e = Rmap[dst]
    t_e = r_e // P
    p_e = r_e % P

    # per-edge region: forced outside the overlap band [RB0, RA); edges whose
    # src rank falls inside it are flex-assigned to balance per-(c,t) counts
    src_c = Cmap[src]
    src_r = Rmap[src]
    forcedA = src_r < RB0
    forcedB = src_r >= RA
    flex = ~(forcedA | forcedB)
    gid = c_e * T + t_e
    nA = np.bincount(gid[forcedA], minlength=C * T)
    nF = np.bincount(gid[flex], minlength=C * T)
    ntot = np.bincount(gid, minlength=C * T)
    addA = np.clip((ntot + 1) // 2 - nA, 0, nF)
    fidx = np.where(flex)[0]
    fg = gid[fidx]
    forder = np.argsort(fg, kind="stable")
    fo = fidx[forder]
    fgo = fg[forder]
    gstarts = np.concatenate(([0], np.cumsum(np.bincount(fgo, minlength=C * T))))[:-1]
    rank_in_g = np.arange(len(fo)) - gstarts[fgo]
    region = np.ones(E, np.int8)
    region[forcedA] = 0
    region[fo[rank_in_g < addA[fgo]]] = 0
    # row index within the half's table: A: c*RA + rank; B: c*RB_ + rank-RB0
    lsrc_half = np.where(region == 0, src_c * RA + src_r,
                         src_c * RB_ + (src_r - RB0))

    # per (c, t, r) counts and shared block counts
    cnt = np.zeros((C, T, 2), np.int64)
    np.add.at(cnt, (c_e, t_e, region), 1)
    nblk = np.maximum(1, (cnt.max(axis=0) + P - 1) // P)  # [T, 2]

    # global block order: superchunk-major, region inner, tile inner
    rounds = []
    binfo = []  # per global block: (tile, region)
    b0_of = {}  # (t, r) -> first global block idx
    gb = 0
    for sc in range(NSC):
        tiles = range(sc * SC, min(T, (sc + 1) * SC))
        for r in range(2):
            run = []
            for t in tiles:
                b0_of[(t, r)] = gb
                run.append((t, int(nblk[t, r])))
                for _ in range(int(nblk[t, r])):
                    binfo.append((t, r))
                gb += int(nblk[t, r])
            # split run into rounds of <= RBCAP blocks
            cur = []
            cb = 0
            curb0 = b0_of[(run[0][0], r)]
            for (t, nb) in run:
                left = nb
                while left > 0:
                    take = min(left, RBCAP - cb)
                    if take == 0:
                        rounds.append((r, curb0, cur))
                        curb0 += cb
                        cur = []
                        cb = 0
                        continue
                    cur.append((t, take))
                    cb += take
                    left -= take
            if cur:
                rounds.append((r, curb0, cur))
    totb = gb

    # place each edge: (global block, row)
    eorder = np.lexsort((r_e, region.astype(np.int64), t_e, c_e))
    # recompute placement per (c,t,r) run using cumulative position
    ce_s = c_e[eorder]
    te_s = t_e[eorder]
    re_s = region[eorder].astype(np.int64)
    key = (ce_s * T + te_s) * 2 + re_s
    kcnt = np.bincount(key, minlength=C * T * 2)
    kstart = np.concatenate(([0], np.cumsum(kcnt)))[:-1]
    pos_in_run = np.arange(E) - kstart[key]
    b0_arr = np.zeros((T, 2), np.int64)
    for t in range(T):
        for r in range(2):
            b0_arr[t, r] = b0_of[(t, r)]
    blk = b0_arr[te_s, re_s] + pos_in_run // P
    row = pos_in_run % P
    loc = lsrc_half[eorder].astype(np.int16)
    pe_s = p_e[eorder]

    # idx array: [totb*128] int16, pad = 0
    iw16 = totb * 8
    idxflat = np.zeros(totb * P, np.int16)
    idxflat[blk * P + row] = loc
    IDX = np.empty((C, P, iw16), np.int16)
    # per-core idx: only this core's edges (others' blocks stay pad)
    for c in range(C):
        m = ce_s == c
        f = np.zeros(totb * P, np.int16)
        f[blk[m] * P + row[m]] = loc[m]
        IDX[c] = _wrap16(f)

    # S matrices, device layout [C, P(row), totb*P]: sst[r, b*P+c] = S[b,r,c]
    S = np.zeros((C, P, totb * P), ml_dtypes.bfloat16)
    one = ml_dtypes.bfloat16(1.0)
    for c in range(C):
        m = ce_s == c
        S[c, row[m], blk[m] * P + pe_s[m]] = one

    return ranked, Lmap, rounds, totb, iw16, IDX, S


def kernel(x, src, dst, perm, W1, b1, a1, W2, b2, a2, Wm, bm):
    x = np.ascontiguousarray(np.asarray(x, np.float32))
    src = np.asarray(src, np.int64)
    dst = np.asarray(dst, np.int64)
    perm = np.asarray(perm, np.int64)
    W1 = np.asarray(W1, np.float32); W2 = np.asarray(W2, np.float32)
    Wm = np.asarray(Wm, np.float32)
    b1 = np.asarray(b1, np.float32); b2 = np.asarray(b2, np.float32)
    a1 = np.asarray(a1, np.float32); a2 = np.asarray(a2, np.float32)
    bm = np.asarray(bm, np.float32)

    deg_out = np.bincount(src, minlength=N)
    deg_in = np.bincount(dst, minlength=N)
    norm_out = np.maximum(deg_out, 1).astype(np.float32) ** -0.5
    norm_in = np.maximum(deg_in, 1).astype(np.float32) ** -0.5

    ranked, Lmap, rounds, totb, iw16, IDX, S = _host_plan(src, dst)

    def tile_pt(vals_pad):
        return np.ascontiguousarray(vals_pad.reshape(C, T, P).transpose(0, 2, 1))

    rmask = ranked >= 0
    rk = np.where(rmask, ranked, 0)
    no_p = np.where(rmask, norm_out[rk], 0).astype(np.float32)
    ni_p = np.where(rmask, norm_in[rk], 0).astype(np.float32)
    NO = tile_pt(no_p); NI = tile_pt(ni_p)

    # norm_out folded into x on the host (both views scale by the table
    # OWNER row's norm_out)
    xp = np.zeros((2, C, NLP, FIN), ml_dtypes.bfloat16)
    for c in range(C):
        m = rmask[c]
        nodes = rk[c][m]
        sc_ = norm_out[nodes][:, None]
        xp[0, c, m] = (x[nodes] * sc_).astype(ml_dtypes.bfloat16)
        xp[1, c, m] = (x[perm[nodes]] * sc_).astype(ml_dtypes.bfloat16)
    # device layout: xt[v, s, k, t*P + p] = x_view_v[node(t,p), s*KSL + k]
    XT = np.ascontiguousarray(
        xp.reshape(2, C, T, P, KS, KSL).transpose(1, 0, 4, 5, 2, 3)
        .reshape(C, 2, KS, KSL, T * P))

    w1s = np.ascontiguousarray(W1.reshape(KS, KSL, H)).astype(ml_dtypes.bfloat16)
    rep = lambda v: np.ascontiguousarray(np.tile(v[None, :], (P, 1)))
    wv = Wm.sum(axis=1)
    wvb = np.concatenate([wv, [bm.sum()]]).astype(np.float32)
    hasb1 = bool(np.any(b1)); hasb2 = bool(np.any(b2))

    rkey = tuple((r, b0, tuple(tl)) for (r, b0, tl) in rounds)
    ck = (hash(rkey), totb, iw16, hasb1, hasb2,
          os.environ.get("KERNEL_DBG", "0"))
    if ck not in _cache:
        _cache[ck] = _build(rounds, totb, iw16, hasb1, hasb2)
    nc = _cache[ck]

    shared = {
        "w1_in": w1s, "w2_in": W2.astype(ml_dtypes.bfloat16),
        "b1_in": rep(b1), "a1_in": rep(a1),
        "b2_in": rep(b2), "a2_in": rep(a2), "wv_in": rep(wvb),
    }
    in_maps = []
    for c in range(C):
        m = dict(shared)
        m.update({
            "xt": XT[c], "idx": IDX[c], "s_in": S[c],
            "no_in": NO[c], "ni_in": NI[c],
        })
        in_maps.append(m)

    trace = os.environ.get("KERNEL_TRACE", "0") == "1"
    res = run_bass_kernel_spmd(nc, in_maps, core_ids=list(range(C)), trace=trace)
    if res.exec_time_ns is not None:
        print(f"HW exec time: {res.exec_time_ns} ns")
        kernel.last_exec_time_ns = res.exec_time_ns

    z = np.empty((2, N), np.float32)
    for c in range(C):
        o = res.results[c]["out"]
        m = rmask[c]
        z[0, ranked[c][m]] = o[0][m]
        z[1, ranked[c][m]] = o[1][m]
    return np.concatenate([z[0], z[1]]).astype(np.float32)

